# revision 2
# baseline (speedup 1.0000x reference)
"""Trainium2 Bass kernel for nn_MessagePassingLayer (bipartite GNN attention
message passing), distributed over 8 NeuronCores.

Strategy:
  - Node tables row-sharded 8 ways (inv: 6250/core padded to 6272, asset:
    1250/core padded to 1280).
  - Phase A (per core): project local shard: M = FF(h), K = h@Wk, V = M@Wv,
    Q = h@Wq.  FF runs feature-major (activations transposed once via PE
    transpose), so chained matmuls need no further transposes and biases are
    per-partition.  K/V shard tables are AllGathered into full tables.
  - Phase B: edges are bucketed on host by *target* (so the segment softmax
    and scatter-add are core-local).  Per 128-edge chunk: dma_gather Q[tgt],
    K[src], V[src] rows; per-edge scores via fused mul+reduce; ex =
    nw*exp(score/8) (max-subtraction dropped -- numerically exact here to
    1e-10 relative, see analysis); one-hot(target) matmuls accumulate
    numer = sum(ex*V) and sumex per target block in PSUM; divide at block end.
  - Phase C: out = FF([h, msg] @ u_w*) on local shards, written sharded.
Host assembles full outputs.
"""

import math
import numpy as np
from contextlib import ExitStack

import concourse.bass as bass
import concourse.tile as tile
from concourse import bacc, mybir
from concourse import bass_utils

F32 = mybir.dt.float32
I16 = mybir.dt.int16
AF = mybir.ActivationFunctionType
ALU = mybir.AluOpType

# ---- problem constants (hardcoded per contest contract) ----
I_N, A_N, E_N = 50000, 10000, 200000
D, H, DK = 256, 4, 64
NC = 8
P = 128
ISH, ASH = I_N // NC, A_N // NC            # 6250, 1250
ISHP, ASHP = 6272, 1280                    # padded to mult of 128
NT_I, NT_A = ISHP // P, ASHP // P          # 49, 10 tiles per core
SPLIT = 32768                              # int16 index ceiling for dma_gather
SUBB = 8                                   # max chunks per gather call

_LAST_EXEC_NS = None


# ----------------------------------------------------------------------------
# Host-side edge preparation
# ----------------------------------------------------------------------------

def _prep_direction(tgt, src, nw, tgt_shard, tgt_shard_pad, n_blocks,
                    src_shard, src_shard_pad, need_split):
    """Bucket edges by (target core, target block); pad chunk counts to be
    identical across cores.  Returns (meta, per-core host arrays).

    meta: list over blocks of (KA, KB) chunk counts (KB=0 when no split).
    per-core arrays: src16 [C*128] (padded src table row ids, int16-ready),
    qidx16 [C*128] (target local row in Q shard), t128 [C*128] (block-local
    target or -1), nwv [C*128] f32.
    """
    core = tgt // tgt_shard
    loc = tgt - core * tgt_shard
    blk = loc // P
    t128_all = loc - blk * P
    # padded source table row
    srow = (src // src_shard) * src_shard_pad + (src % src_shard)

    # group edge indices per (core, block, seg)
    buckets = {}
    for c in range(NC):
        m_c = core == c
        for b in range(n_blocks):
            m = m_c & (blk == b)
            idx = np.nonzero(m)[0]
            if need_split:
                sa = idx[srow[idx] < SPLIT]
                sb = idx[srow[idx] >= SPLIT]
                buckets[(c, b)] = (sa, sb)
            else:
                buckets[(c, b)] = (idx, idx[:0])

    meta = []
    for b in range(n_blocks):
        ka = max((len(buckets[(c, b)][0]) + P - 1) // P for c in range(NC))
        kb = max((len(buckets[(c, b)][1]) + P - 1) // P for c in range(NC))
        meta.append((ka, kb))
    C = sum(ka + kb for ka, kb in meta)

    cores_out = []
    for c in range(NC):
        src16 = np.zeros(C * P, np.int64)
        qidx16 = np.zeros(C * P, np.int64)
        t128 = np.full(C * P, -1.0, np.float32)
        nwv = np.zeros(C * P, np.float32)
        pos = 0
        for b in range(n_blocks):
            ka, kb = meta[b]
            for seg_i, klen in ((0, ka), (1, kb)):
                idx = buckets[(c, b)][seg_i]
                n = len(idx)
                if klen == 0:
                    continue
                sl = slice(pos, pos + n)
                sr = srow[idx]
                if seg_i == 1:
                    sr = sr - SPLIT
                src16[sl] = sr
                qidx16[sl] = loc[idx]
                t128[sl] = t128_all[idx].astype(np.float32)
                nwv[sl] = nw[idx]
                pos += klen * P
        assert pos == C * P
        cores_out.append((src16, qidx16, t128, nwv))
    return meta, C, cores_out


def _wrap16(flat_idx, C):
    """[C*128] int -> dma_gather idx layout [128, C*8] int16 (16-wrapped,
    replicated across the 8 gpsimd core groups)."""
    assert flat_idx.max(initial=0) < SPLIT and flat_idx.min(initial=0) >= 0
    w = flat_idx.astype(np.int16).reshape(C * 8, 16).T  # [16, C*8]
    return np.tile(w, (8, 1)).copy()                    # [128, C*8]


def _colmajor(flat, C, rep=1):
    """[C*128] -> [128, C] (partition = edge%128, col = chunk)."""
    a = flat.reshape(C, P).T.copy()                     # [128, C]
    if rep == 1:
        return a
    return np.repeat(a[:, :, None], rep, axis=2).copy()  # [128, C, rep]


def _pad_rows(a, n):
    out = np.zeros((n, a.shape[1]), a.dtype)
    out[: a.shape[0]] = a
    return out


# ----------------------------------------------------------------------------
# Device program
# ----------------------------------------------------------------------------

def _build(meta1, C1, meta2, C2):
    nc = bacc.Bacc("TRN2", target_bir_lowering=False, debug=False,
                   enable_asserts=True, num_devices=NC)

    # ---- IO ----
    inv_h = nc.dram_tensor("inv_h", [ISHP, D], F32, kind="ExternalInput")
    ast_h = nc.dram_tensor("ast_h", [ASHP, D], F32, kind="ExternalInput")
    w_m1 = nc.dram_tensor("w_m1", [2, P, D], F32, kind="ExternalInput")
    w_m2 = nc.dram_tensor("w_m2", [2, P, D], F32, kind="ExternalInput")
    w_q = nc.dram_tensor("w_q", [2, P, D], F32, kind="ExternalInput")
    w_k = nc.dram_tensor("w_k", [2, P, D], F32, kind="ExternalInput")
    w_v = nc.dram_tensor("w_v", [2, P, D], F32, kind="ExternalInput")
    w_u1 = nc.dram_tensor("w_u1", [4, P, D], F32, kind="ExternalInput")
    w_u2 = nc.dram_tensor("w_u2", [2, P, D], F32, kind="ExternalInput")
    b_m1 = nc.dram_tensor("b_m1", [P, 2], F32, kind="ExternalInput")
    b_m2 = nc.dram_tensor("b_m2", [P, 2], F32, kind="ExternalInput")
    b_u1 = nc.dram_tensor("b_u1", [P, 2], F32, kind="ExternalInput")
    b_u2r = nc.dram_tensor("b_u2r", [1, D], F32, kind="ExternalInput")
    iota_in = nc.dram_tensor("iota_in", [P, P], F32, kind="ExternalInput")
    ident_in = nc.dram_tensor("ident_in", [P, P], F32, kind="ExternalInput")

    d1_src = nc.dram_tensor("d1_src", [P, C1 * 8], I16, kind="ExternalInput")
    d1_q = nc.dram_tensor("d1_q", [P, C1 * 8], I16, kind="ExternalInput")
    d1_t = nc.dram_tensor("d1_t", [P, C1], F32, kind="ExternalInput")
    d1_nw = nc.dram_tensor("d1_nw", [P, C1, H], F32, kind="ExternalInput")
    d2_src = nc.dram_tensor("d2_src", [P, C2 * 8], I16, kind="ExternalInput")
    d2_q = nc.dram_tensor("d2_q", [P, C2 * 8], I16, kind="ExternalInput")
    d2_t = nc.dram_tensor("d2_t", [P, C2], F32, kind="ExternalInput")
    d2_nw = nc.dram_tensor("d2_nw", [P, C2, H], F32, kind="ExternalInput")

    out_inv = nc.dram_tensor("out_inv", [ISHP, D], F32, kind="ExternalOutput")
    out_ast = nc.dram_tensor("out_ast", [ASHP, D], F32, kind="ExternalOutput")

    with tile.TileContext(nc) as tc:
        with ExitStack() as ctx:
            wpool = ctx.enter_context(tc.tile_pool(name="w", bufs=1))
            hp = ctx.enter_context(tc.tile_pool(name="hp", bufs=3))
            tp = ctx.enter_context(tc.tile_pool(name="tp", bufs=3))
            op = ctx.enter_context(tc.tile_pool(name="op", bufs=3))
            gp = ctx.enter_context(tc.tile_pool(name="gp", bufs=2))
            sp = ctx.enter_context(tc.tile_pool(name="sp", bufs=4))
            ohp = ctx.enter_context(tc.tile_pool(name="ohp", bufs=3))
            ps_t = ctx.enter_context(tc.tile_pool(name="ps_t", bufs=2, space="PSUM"))
            ps_mm = ctx.enter_context(tc.tile_pool(name="ps_mm", bufs=2, space="PSUM"))
            ps_nu = ctx.enter_context(tc.tile_pool(name="ps_nu", bufs=2, space="PSUM"))
            ps_se = ctx.enter_context(tc.tile_pool(name="ps_se", bufs=2, space="PSUM"))
            dram = ctx.enter_context(tc.tile_pool(name="dram", bufs=1, space="DRAM"))

            # ---- constants / weights resident in SBUF ----
            iota_t = wpool.tile([P, P], F32, tag="iota_t")
            nc.sync.dma_start(iota_t[:], iota_in[:, :])
            ident = wpool.tile([P, P], F32, tag="ident")
            nc.sync.dma_start(ident[:], ident_in[:, :])
            ones_t = wpool.tile([1, P], F32, tag="ones_t")
            nc.vector.memset(ones_t[:], 1.0)

            def load_w(dram_w, nk, tag):
                t = wpool.tile([P, nk, D], F32, tag=tag)
                for k in range(nk):
                    nc.sync.dma_start(t[:, k, :], dram_w[k, :, :])
                return t

            m1_t = load_w(w_m1, 2, "wm1")
            m2_t = load_w(w_m2, 2, "wm2")
            q_t = load_w(w_q, 2, "wq")
            k_t = load_w(w_k, 2, "wk")
            v_t = load_w(w_v, 2, "wv")
            u1_t = load_w(w_u1, 4, "wu1")
            u2_t = load_w(w_u2, 2, "wu2")

            def load_b(dram_b, tag):
                t = wpool.tile([P, 2], F32, tag=tag)
                nc.sync.dma_start(t[:], dram_b[:, :])
                return t

            bm1_t, bm2_t, bu1_t = load_b(b_m1, "bm1"), load_b(b_m2, "bm2"), load_b(b_u1, "bu1")
            bu2_t = wpool.tile([1, D], F32, tag="bu2")
            nc.sync.dma_start(bu2_t[:], b_u2r[:, :])

            # ---- DRAM scratch ----
            q_inv = dram.tile([ISHP, D], F32, tag="q_inv")
            q_ast = dram.tile([ASHP, D], F32, tag="q_ast")
            hT_inv = dram.tile([NT_I, 2, P, P], F32, tag="hT_inv")
            hT_ast = dram.tile([NT_A, 2, P, P], F32, tag="hT_ast")
            k_i_sh = dram.tile([ISHP, D], F32, tag="k_i_sh")
            v_i_sh = dram.tile([ISHP, D], F32, tag="v_i_sh")
            k_a_sh = dram.tile([ASHP, D], F32, tag="k_a_sh")
            v_a_sh = dram.tile([ASHP, D], F32, tag="v_a_sh")
            k_i_full = dram.tile([ISHP * NC, D], F32, tag="k_i_full")
            v_i_full = dram.tile([ISHP * NC, D], F32, tag="v_i_full")
            k_a_full = dram.tile([ASHP * NC, D], F32, tag="k_a_full")
            v_a_full = dram.tile([ASHP * NC, D], F32, tag="v_a_full")
            msg_inv = dram.tile([ISHP, D], F32, tag="msg_inv")
            msg_ast = dram.tile([ASHP, D], F32, tag="msg_ast")

            # ================= Phase A: projections =================
            def phase_a(h_dram, ntiles, hT_dram, q_dram, k_sh, v_sh):
                for t in range(ntiles):
                    ht = hp.tile([P, D], F32, tag="ht")
                    nc.sync.dma_start(ht[:], h_dram[t * P:(t + 1) * P, :])
                    hT = tp.tile([P, 2, P], F32, tag="hT")
                    for k in range(2):
                        pt = ps_t.tile([P, P], F32, tag="pt")
                        nc.tensor.transpose(pt[:], ht[:, k * P:(k + 1) * P], ident[:])
                        nc.vector.tensor_copy(hT[:, k, :], pt[:])
                    nc.sync.dma_start(
                        hT_dram[t].transpose([1, 0, 2]), hT[:])

                    # FF chain, feature-major
                    def ff_layer(inT, w_tile, b_tile, tag):
                        outT = tp.tile([P, 2, P], F32, tag=tag)
                        for hf in range(2):
                            pm = ps_mm.tile([P, D], F32, tag="pm")
                            for k in range(2):
                                nc.tensor.matmul(
                                    pm[:, 0:P],
                                    lhsT=w_tile[:, k, hf * P:(hf + 1) * P],
                                    rhs=inT[:, k, :],
                                    start=(k == 0), stop=(k == 1))
                            nc.scalar.activation(
                                outT[:, hf, :], pm[:, 0:P], AF.Gelu,
                                bias=b_tile[:, hf:hf + 1])
                        return outT

                    mT1 = ff_layer(hT, m1_t, bm1_t, "mT1")
                    mT2 = ff_layer(mT1, m2_t, bm2_t, "mT2")

                    # node-major tables
                    for w_tile, dest, lT in (
                        (q_t, q_dram, hT), (k_t, k_sh, hT), (v_t, v_sh, mT2)
                    ):
                        pm = ps_mm.tile([P, D], F32, tag="pm")
                        for k in range(2):
                            nc.tensor.matmul(
                                pm[:], lhsT=lT[:, k, :], rhs=w_tile[:, k, :],
                                start=(k == 0), stop=(k == 1))
                        ot = op.tile([P, D], F32, tag="proj")
                        nc.vector.tensor_copy(ot[:], pm[:])
                        nc.sync.dma_start(dest[t * P:(t + 1) * P, :], ot[:])

            phase_a(ast_h, NT_A, hT_ast, q_ast, k_a_sh, v_a_sh)
            phase_a(inv_h, NT_I, hT_inv, q_inv, k_i_sh, v_i_sh)

            # ================= AllGather K/V tables =================
            rg = [list(range(NC))]
            for src, dst in ((k_a_sh, k_a_full), (v_a_sh, v_a_full),
                             (k_i_sh, k_i_full), (v_i_sh, v_i_full)):
                nc.gpsimd.collective_compute(
                    "AllGather", ALU.bypass, replica_groups=rg,
                    ins=[src.opt()], outs=[dst.opt()])

            # ================= Phase B: edge attention =================
            def phase_b(meta, src_sb, q_sb, t_sb, nw_sb, q_dram, k_full,
                        v_full, msg_dram):
                cg = 0
                for b, (ka, kb) in enumerate(meta):
                    K = ka + kb
                    if K == 0:
                        z = op.tile([P, D], F32, tag="msg")
                        nc.vector.memset(z[:], 0.0)
                        nc.sync.dma_start(msg_dram[b * P:(b + 1) * P, :], z[:])
                        continue
                    numer = ps_nu.tile([P, D], F32, tag="nu")
                    sumex = ps_se.tile([P, H], F32, tag="se")
                    nchunk = 0
                    for seg_start, seg_len, base in (
                        (cg, ka, 0), (cg + ka, kb, SPLIT)
                    ):
                        for s0 in range(0, seg_len, SUBB):
                            n = min(SUBB, seg_len - s0)
                            j0 = seg_start + s0
                            qg = gp.tile([P, SUBB, D], F32, tag="qg")
                            kg = gp.tile([P, SUBB, D], F32, tag="kg")
                            vg = gp.tile([P, SUBB, D], F32, tag="vg")
                            for g_out, g_tbl, g_idx in (
                                (qg, q_dram[:, :], q_sb),
                                (kg, k_full[base:, :], src_sb),
                                (vg, v_full[base:, :], src_sb),
                            ):
                                nc.gpsimd.dma_gather(
                                    out_ap=g_out[:, 0:n, :],
                                    in_ap=g_tbl,
                                    idxs_ap=g_idx[:, j0 * 8:(j0 + n) * 8],
                                    num_idxs=n * P,
                                    num_idxs_reg=n * P,
                                    elem_size=D)
                            for j in range(n):
                                c = j0 + j
                                oh = ohp.tile([P, P], F32, tag="oh")
                                nc.vector.tensor_scalar(
                                    oh[:], iota_t[:], t_sb[:, c:c + 1], None,
                                    ALU.is_equal)
                                s4 = sp.tile([P, H], F32, tag="s4")
                                qk = sp.tile([P, D], F32, tag="qk")
                                for h in range(H):
                                    nc.vector.scalar_tensor_tensor(
                                        out=qk[:, h * DK:(h + 1) * DK],
                                        in0=qg[:, j, h * DK:(h + 1) * DK],
                                        scalar=0.125,
                                        in1=kg[:, j, h * DK:(h + 1) * DK],
                                        op0=ALU.mult, op1=ALU.mult,
                                        accum_out=s4[:, h:h + 1])
                                ex0 = sp.tile([P, H], F32, tag="ex0")
                                nc.scalar.activation(ex0[:], s4[:], AF.Exp)
                                ex = sp.tile([P, H], F32, tag="ex")
                                nc.vector.tensor_tensor(
                                    ex[:], ex0[:], nw_sb[:, c, :], ALU.mult)
                                exv = sp.tile([P, D], F32, tag="exv")
                                for h in range(H):
                                    nc.vector.tensor_scalar(
                                        exv[:, h * DK:(h + 1) * DK],
                                        vg[:, j, h * DK:(h + 1) * DK],
                                        ex[:, h:h + 1], None, ALU.mult)
                                first = nchunk == 0
                                last = nchunk == K - 1
                                nc.tensor.matmul(numer[:], lhsT=oh[:], rhs=exv[:],
                                                 start=first, stop=last)
                                nc.tensor.matmul(sumex[:], lhsT=oh[:], rhs=ex[:],
                                                 start=first, stop=last)
                                nchunk += 1
                    # finalize block
                    den = sp.tile([P, H], F32, tag="den")
                    nc.vector.tensor_scalar(den[:], sumex[:], 1e-10, None, ALU.add)
                    rec = sp.tile([P, H], F32, tag="rec")
                    nc.vector.reciprocal(rec[:], den[:])
                    msg = op.tile([P, D], F32, tag="msg")
                    for h in range(H):
                        nc.vector.tensor_scalar(
                            msg[:, h * DK:(h + 1) * DK],
                            numer[:, h * DK:(h + 1) * DK],
                            rec[:, h:h + 1], None, ALU.mult)
                    nc.sync.dma_start(msg_dram[b * P:(b + 1) * P, :], msg[:])
                    cg += K

            # load edge metadata into SBUF
            d1_src_sb = wpool.tile([P, C1 * 8], I16, tag="d1_src_sb")
            nc.sync.dma_start(d1_src_sb[:], d1_src[:, :])
            d1_q_sb = wpool.tile([P, C1 * 8], I16, tag="d1_q_sb")
            nc.sync.dma_start(d1_q_sb[:], d1_q[:, :])
            d1_t_sb = wpool.tile([P, C1], F32, tag="d1_t_sb")
            nc.sync.dma_start(d1_t_sb[:], d1_t[:, :])
            d1_nw_sb = wpool.tile([P, C1, H], F32, tag="d1_nw_sb")
            nc.sync.dma_start(d1_nw_sb[:], d1_nw[:, :, :])
            d2_src_sb = wpool.tile([P, C2 * 8], I16, tag="d2_src_sb")
            nc.sync.dma_start(d2_src_sb[:], d2_src[:, :])
            d2_q_sb = wpool.tile([P, C2 * 8], I16, tag="d2_q_sb")
            nc.sync.dma_start(d2_q_sb[:], d2_q[:, :])
            d2_t_sb = wpool.tile([P, C2], F32, tag="d2_t_sb")
            nc.sync.dma_start(d2_t_sb[:], d2_t[:, :])
            d2_nw_sb = wpool.tile([P, C2, H], F32, tag="d2_nw_sb")
            nc.sync.dma_start(d2_nw_sb[:], d2_nw[:, :, :])

            phase_b(meta1, d1_src_sb, d1_q_sb, d1_t_sb, d1_nw_sb,
                    q_inv, k_a_full, v_a_full, msg_inv)
            phase_b(meta2, d2_src_sb, d2_q_sb, d2_t_sb, d2_nw_sb,
                    q_ast, k_i_full, v_i_full, msg_ast)

            # ================= Phase C: update FF =================
            def phase_c(ntiles, hT_dram, msg_dram, out_dram):
                for t in range(ntiles):
                    msg = hp.tile([P, D], F32, tag="cmsg")
                    nc.sync.dma_start(msg[:], msg_dram[t * P:(t + 1) * P, :])
                    cat = tp.tile([P, 4, P], F32, tag="cat")
                    for k in range(2):
                        nc.sync.dma_start(cat[:, k, :], hT_dram[t, k])
                        pt = ps_t.tile([P, P], F32, tag="pt")
                        nc.tensor.transpose(pt[:], msg[:, k * P:(k + 1) * P], ident[:])
                        nc.vector.tensor_copy(cat[:, 2 + k, :], pt[:])
                    y1 = tp.tile([P, 2, P], F32, tag="y1")
                    for hf in range(2):
                        pm = ps_mm.tile([P, D], F32, tag="pm")
                        for k in range(4):
                            nc.tensor.matmul(
                                pm[:, 0:P],
                                lhsT=u1_t[:, k, hf * P:(hf + 1) * P],
                                rhs=cat[:, k, :],
                                start=(k == 0), stop=(k == 3))
                        nc.scalar.activation(
                            y1[:, hf, :], pm[:, 0:P], AF.Gelu,
                            bias=bu1_t[:, hf:hf + 1])
                    po = ps_mm.tile([P, D], F32, tag="pm")
                    for k in range(2):
                        nc.tensor.matmul(po[:], lhsT=y1[:, k, :], rhs=u2_t[:, k, :],
                                         start=(k == 0), stop=False)
                    nc.tensor.matmul(po[:], lhsT=ones_t[0:1, :], rhs=bu2_t[0:1, :],
                                     start=False, stop=True)
                    ot = op.tile([P, D], F32, tag="fin")
                    nc.scalar.activation(ot[:], po[:], AF.Gelu)
                    nc.sync.dma_start(out_dram[t * P:(t + 1) * P, :], ot[:])

            phase_c(NT_I, hT_inv, msg_inv, out_inv)
            phase_c(NT_A, hT_ast, msg_ast, out_ast)

    nc.compile()
    return nc


# ----------------------------------------------------------------------------
# Entry point
# ----------------------------------------------------------------------------

def kernel(inv_h, asset_h, inv_norm_w, asset_norm_w,
           m_w1, m_b1, m_w2, m_b2, Wq, Wk, Wv,
           u_w1, u_b1, u_w2, u_b2, edge_tgt, edge_src):
    global _LAST_EXEC_NS
    inv_h = np.asarray(inv_h, np.float32)
    asset_h = np.asarray(asset_h, np.float32)
    inv_norm_w = np.asarray(inv_norm_w, np.float32)
    asset_norm_w = np.asarray(asset_norm_w, np.float32)
    edge_tgt = np.asarray(edge_tgt).astype(np.int64)
    edge_src = np.asarray(edge_src).astype(np.int64)
    m_w1 = np.asarray(m_w1, np.float32)
    m_b1 = np.asarray(m_b1, np.float32)
    m_w2 = np.asarray(m_w2, np.float32)
    m_b2 = np.asarray(m_b2, np.float32)
    Wq = np.asarray(Wq, np.float32)
    Wk = np.asarray(Wk, np.float32)
    Wv = np.asarray(Wv, np.float32)
    u_w1 = np.asarray(u_w1, np.float32)
    u_b1 = np.asarray(u_b1, np.float32)
    u_w2 = np.asarray(u_w2, np.float32)
    u_b2 = np.asarray(u_b2, np.float32)

    # dir1: targets = investors, sources = assets
    meta1, C1, d1 = _prep_direction(
        edge_tgt, edge_src, inv_norm_w,
        ISH, ISHP, NT_I, ASH, ASHP, need_split=False)
    # dir2: targets = assets, sources = investors
    meta2, C2, d2 = _prep_direction(
        edge_src, edge_tgt, asset_norm_w,
        ASH, ASHP, NT_A, ISH, ISHP, need_split=True)

    nc = _build(meta1, C1, meta2, C2)

    iota = np.broadcast_to(np.arange(P, dtype=np.float32), (P, P)).copy()
    ident = np.eye(P, dtype=np.float32)

    common = {
        "w_m1": m_w1.reshape(2, P, D).copy(),
        "w_m2": m_w2.reshape(2, P, D).copy(),
        "w_q": Wq.reshape(2, P, D).copy(),
        "w_k": Wk.reshape(2, P, D).copy(),
        "w_v": Wv.reshape(2, P, D).copy(),
        "w_u1": u_w1.reshape(4, P, D).copy(),
        "w_u2": u_w2.reshape(2, P, D).copy(),
        "b_m1": m_b1.reshape(2, P).T.copy(),
        "b_m2": m_b2.reshape(2, P).T.copy(),
        "b_u1": u_b1.reshape(2, P).T.copy(),
        "b_u2r": u_b2.reshape(1, D).copy(),
        "iota_in": iota,
        "ident_in": ident,
    }

    in_maps = []
    for c in range(NC):
        s1, q1, t1, n1 = d1[c]
        s2, q2, t2, n2 = d2[c]
        m = dict(common)
        m["inv_h"] = _pad_rows(inv_h[c * ISH:(c + 1) * ISH], ISHP)
        m["ast_h"] = _pad_rows(asset_h[c * ASH:(c + 1) * ASH], ASHP)
        m["d1_src"] = _wrap16(s1, C1)
        m["d1_q"] = _wrap16(q1, C1)
        m["d1_t"] = _colmajor(t1, C1)
        m["d1_nw"] = _colmajor(n1, C1, rep=H)
        m["d2_src"] = _wrap16(s2, C2)
        m["d2_q"] = _wrap16(q2, C2)
        m["d2_t"] = _colmajor(t2, C2)
        m["d2_nw"] = _colmajor(n2, C2, rep=H)
        in_maps.append(m)

    res = bass_utils.run_bass_kernel_spmd(
        nc, in_maps, core_ids=list(range(NC)), trace=True)
    _LAST_EXEC_NS = res.exec_time_ns

    inv_out = np.concatenate(
        [res.results[c]["out_inv"][:ISH] for c in range(NC)], axis=0)
    ast_out = np.concatenate(
        [res.results[c]["out_ast"][:ASH] for c in range(NC)], axis=0)
    return inv_out, ast_out


# revision 9
# speedup vs baseline: 1.2356x; 1.2356x over previous
"""Trainium2 Bass kernel for nn_MessagePassingLayer (bipartite GNN attention
message passing), distributed over 8 NeuronCores.

v5 design notes:
  - Node tables row-sharded 8 ways (inv 6250->6272 padded, asset 1250->1280).
  - dma_gather on TRN2 is descriptor-generation bound (~5.9ns/idx + 2.6us/call,
    independent of row bytes), so K|V are packed into one fp32 [N,512] row
    (one gather per edge total) and Q is never gathered: per target block the
    128 Q rows are densely loaded and Qe is formed on the PE with a
    host-precomputed transposed one-hot (exact in bf16).
  - Phase A: project local shard; FF runs feature-major off a host-supplied
    transposed bf16 copy of h, so there are no on-device transposes.  Small
    asset tables (kv_a fp32, q_a bf16) are AllGathered; the 51MB investor
    tables stay local (dir2 is shard-by-source + ReduceScatter of the segment
    stats instead).
  - Phase B per 128-edge chunk: gather KV[src]; Qe = oh_te @ Qblk (PE);
    qk = Qe*Ke (DVE, fp32); per-head reduce; ex = nw*exp(s/8) (max-term
    dropped -- exact to 1e-10 rel here: ex equals the reference's exactly
    when the segment max < 0, and the 1e-10 eps is negligible otherwise);
    exe=[ex*V | ex] in bf16; numer|sumex accumulated per target block by one
    one-hot matmul into fp32 PSUM; divide at block end.  Batched DVE ops
    amortize op overhead across 8-chunk gather batches.
  - dir2 partial numer/sumex (over all 10240 asset rows) are ReduceScattered.
  - Phase C: out = FF([h, msg]); msg transposed via hardware DMA-transpose
    (bf16); matmuls bf16, final gelu fp32 out.
  - gelu and exp live in different ACT table sets; an explicit dep keeps all
    phase-C gelus after the last phase-B exp to avoid table-reload thrash.
"""

import ml_dtypes
import numpy as np
from contextlib import ExitStack

import concourse.bass as bass
import concourse.tile as tile
from concourse.tile import add_dep_helper
from concourse import bacc, mybir
from concourse import bass_utils

F32 = mybir.dt.float32
BF16 = mybir.dt.bfloat16
I16 = mybir.dt.int16
AF = mybir.ActivationFunctionType
ALU = mybir.AluOpType

I_N, A_N, E_N = 50000, 10000, 200000
D, H, DK = 256, 4, 64
D2 = 2 * D
NC = 8
P = 128
ISH, ASH = I_N // NC, A_N // NC            # 6250, 1250
ISHP, ASHP = 6272, 1280
NT_I, NT_A = ISHP // P, ASHP // P          # 49, 10
NB2 = ASHP * NC // P                       # 80 global asset blocks
SUBB = 8                                   # chunks per gather call (1024 idx cap)

_LAST_EXEC_NS = None


# ----------------------------------------------------------------------------
# Host-side edge preparation
# ----------------------------------------------------------------------------

def _bucket(core, blk, n_blocks, srcidx, t128_all, nw):
    buckets = {}
    for c in range(NC):
        m_c = core == c
        for b in range(n_blocks):
            buckets[(c, b)] = np.nonzero(m_c & (blk == b))[0]
    meta = [max((len(buckets[(c, b)]) + P - 1) // P for c in range(NC))
            for b in range(n_blocks)]
    C = sum(meta)
    cores_out = []
    for c in range(NC):
        s16 = np.zeros(C * P, np.int64)
        t128 = np.full(C * P, -1, np.int64)
        nwv = np.zeros(C * P, np.float32)
        pos = 0
        for b in range(n_blocks):
            idx = buckets[(c, b)]
            n = len(idx)
            if meta[b] == 0:
                continue
            sl = slice(pos, pos + n)
            s16[sl] = srcidx[idx]
            t128[sl] = t128_all[idx]
            nwv[sl] = nw[idx]
            pos += meta[b] * P
        assert pos == C * P
        cores_out.append((s16, t128, nwv))
    return meta, C, cores_out


def _wrap16(flat_idx, C):
    assert flat_idx.max(initial=0) < 32768 and flat_idx.min(initial=0) >= 0
    w = flat_idx.astype(np.int16).reshape(C * 8, 16).T
    return np.tile(w, (8, 1)).copy()


def _colmajor(flat, C, rep=1):
    a = flat.reshape(C, P).T.copy()
    if rep == 1:
        return a
    return np.repeat(a[:, :, None], rep, axis=2).copy()


def _onehots(t128, C):
    """[C*128] targets (-1=pad) -> oh_et [C,128e,128t], oh_te [C,128t,128e]."""
    bf = ml_dtypes.bfloat16
    t = t128.reshape(C, P, 1)
    eq = t == np.arange(P).reshape(1, 1, P)
    oh_et = eq.astype(bf)
    oh_te = np.ascontiguousarray(oh_et.transpose(0, 2, 1))
    return oh_et, oh_te


def _pad_rows(a, n):
    out = np.zeros((n, a.shape[1]), a.dtype)
    out[: a.shape[0]] = a
    return out


def _hT(h_pad, ntiles):
    """[N,256] f32 -> bf16 transposed-tile layout [N,256]:
    row (t*128+f), col (k*128+n) = h[t*128+n, k*128+f]."""
    bf = ml_dtypes.bfloat16
    x = h_pad.reshape(ntiles, P, 2, P)          # (t, n, k, f)
    x = x.transpose(0, 3, 2, 1)                 # (t, f, k, n)
    return np.ascontiguousarray(x.reshape(ntiles * P, D).astype(bf))


def _chunk_info(meta):
    info = []
    for b, k in enumerate(meta):
        for j in range(k):
            info.append((b, j == 0, j == k - 1))
    return info


# ----------------------------------------------------------------------------
# Device program
# ----------------------------------------------------------------------------

def _build(meta1, C1, meta2, C2):
    nc = bacc.Bacc("TRN2", target_bir_lowering=False, debug=False,
                   enable_asserts=True, num_devices=NC)

    hT_i_in = nc.dram_tensor("hT_i_in", [NT_I * P, D], BF16, kind="ExternalInput")
    hT_a_in = nc.dram_tensor("hT_a_in", [NT_A * P, D], BF16, kind="ExternalInput")
    w_m1 = nc.dram_tensor("w_m1", [2, P, D], BF16, kind="ExternalInput")
    w_m2 = nc.dram_tensor("w_m2", [2, P, D], BF16, kind="ExternalInput")
    w_qk = nc.dram_tensor("w_qk", [2, P, D2], BF16, kind="ExternalInput")
    w_v = nc.dram_tensor("w_v", [2, P, D], BF16, kind="ExternalInput")
    w_u1 = nc.dram_tensor("w_u1", [4, P, D], BF16, kind="ExternalInput")
    w_u2 = nc.dram_tensor("w_u2", [2, P, D], BF16, kind="ExternalInput")
    b_m1 = nc.dram_tensor("b_m1", [P, 2], F32, kind="ExternalInput")
    b_m2 = nc.dram_tensor("b_m2", [P, 2], F32, kind="ExternalInput")
    b_u1 = nc.dram_tensor("b_u1", [P, 2], F32, kind="ExternalInput")
    b_u2r = nc.dram_tensor("b_u2r", [1, D], BF16, kind="ExternalInput")

    d1_src = nc.dram_tensor("d1_src", [P, C1 * 8], I16, kind="ExternalInput")
    d1_nw = nc.dram_tensor("d1_nw", [P, C1, H], F32, kind="ExternalInput")
    d1_oet = nc.dram_tensor("d1_oet", [C1, P, P], BF16, kind="ExternalInput")
    d1_ote = nc.dram_tensor("d1_ote", [C1, P, P], BF16, kind="ExternalInput")
    d2_src = nc.dram_tensor("d2_src", [P, C2 * 8], I16, kind="ExternalInput")
    d2_nw = nc.dram_tensor("d2_nw", [P, C2, H], F32, kind="ExternalInput")
    d2_oet = nc.dram_tensor("d2_oet", [C2, P, P], BF16, kind="ExternalInput")
    d2_ote = nc.dram_tensor("d2_ote", [C2, P, P], BF16, kind="ExternalInput")

    out_inv = nc.dram_tensor("out_inv", [ISHP, D], F32, kind="ExternalOutput")
    out_ast = nc.dram_tensor("out_ast", [ASHP, D], F32, kind="ExternalOutput")

    info1 = _chunk_info(meta1)
    info2 = _chunk_info(meta2)

    with tile.TileContext(nc) as tc:
        with ExitStack() as ctx:
            wpool = ctx.enter_context(tc.tile_pool(name="w", bufs=1))
            hp = ctx.enter_context(tc.tile_pool(name="hp", bufs=3))
            tp = ctx.enter_context(tc.tile_pool(name="tp", bufs=3))
            op = ctx.enter_context(tc.tile_pool(name="op", bufs=3))
            gp = ctx.enter_context(tc.tile_pool(name="gp", bufs=2))
            sp = ctx.enter_context(tc.tile_pool(name="sp", bufs=2))
            ohp = ctx.enter_context(tc.tile_pool(name="ohp", bufs=4))
            qbp = ctx.enter_context(tc.tile_pool(name="qbp", bufs=2))
            ps_mm = ctx.enter_context(tc.tile_pool(name="ps_mm", bufs=2, space="PSUM"))
            ps_nu = ctx.enter_context(tc.tile_pool(name="ps_nu", bufs=2, space="PSUM"))
            ps_qe = ctx.enter_context(tc.tile_pool(name="ps_qe", bufs=3, space="PSUM"))
            dram = ctx.enter_context(tc.tile_pool(name="dram", bufs=1, space="DRAM"))

            ones_t = wpool.tile([1, P], BF16, tag="ones_t")
            nc.vector.memset(ones_t[:], 1.0)
            z256 = wpool.tile([P, D], F32, tag="z256")
            nc.vector.memset(z256[:], 0.0)
            z4 = wpool.tile([P, H], F32, tag="z4")
            nc.vector.memset(z4[:], 0.0)
            z256b = wpool.tile([P, D], BF16, tag="z256b")
            nc.vector.memset(z256b[:], 0.0)

            def load_w(dram_w, nk, nd, dt, tag):
                t = wpool.tile([P, nk, nd], dt, tag=tag)
                for k in range(nk):
                    nc.sync.dma_start(t[:, k, :], dram_w[k, :, :])
                return t

            m1_t = load_w(w_m1, 2, D, BF16, "wm1")
            m2_t = load_w(w_m2, 2, D, BF16, "wm2")
            qk_t = load_w(w_qk, 2, D2, BF16, "wqk")
            v_t = load_w(w_v, 2, D, BF16, "wv")
            u1_t = load_w(w_u1, 4, D, BF16, "wu1")
            u2_t = load_w(w_u2, 2, D, BF16, "wu2")

            def load_b(dram_b, tag):
                t = wpool.tile([P, 2], F32, tag=tag)
                nc.sync.dma_start(t[:], dram_b[:, :])
                return t

            bm1_t, bm2_t, bu1_t = load_b(b_m1, "bm1"), load_b(b_m2, "bm2"), load_b(b_u1, "bu1")
            bu2_t = wpool.tile([1, D], BF16, tag="bu2")
            nc.sync.dma_start(bu2_t[:], b_u2r[:, :])

            q_inv = dram.tile([ISHP, D], BF16, tag="q_inv")
            kv_inv = dram.tile([ISHP, D2], F32, tag="kv_inv")
            q_a_sh = dram.tile([ASHP, D], BF16, tag="q_a_sh")
            kv_a_sh = dram.tile([ASHP, D2], F32, tag="kv_a_sh")
            q_a_full = dram.tile([ASHP * NC, D], BF16, tag="q_a_full")
            kv_a_full = dram.tile([ASHP * NC, D2], F32, tag="kv_a_full")
            numer_d = dram.tile([ASHP * NC, D], F32, tag="numer_d")
            sumex_d = dram.tile([ASHP * NC, H], F32, tag="sumex_d")
            numer_sh = dram.tile([ASHP, D], F32, tag="numer_sh")
            sumex_sh = dram.tile([ASHP, H], F32, tag="sumex_sh")
            msg_inv = dram.tile([ISHP, D], BF16, tag="msg_inv")
            msg_ast = dram.tile([ASHP, D], BF16, tag="msg_ast")

            # ================= Phase A =================
            def phase_a(hT_in, ntiles, q_dram, kv_dram):
                for t in range(ntiles):
                    hTb = tp.tile([P, 2, P], BF16, tag="hTb")
                    nc.sync.dma_start(hTb[:], hT_in[t * P:(t + 1) * P, :])

                    def ff_layer(inT, w_tile, b_tile, tag):
                        outT = tp.tile([P, 2, P], BF16, tag=tag)
                        for hf in range(2):
                            pm = ps_mm.tile([P, D2], F32, tag="pm")
                            for k in range(2):
                                nc.tensor.matmul(
                                    pm[:, 0:P],
                                    lhsT=w_tile[:, k, hf * P:(hf + 1) * P],
                                    rhs=inT[:, k, :],
                                    start=(k == 0), stop=(k == 1))
                            nc.scalar.activation(
                                outT[:, hf, :], pm[:, 0:P], AF.Gelu,
                                bias=b_tile[:, hf:hf + 1])
                        return outT

                    mT1 = ff_layer(hTb, m1_t, bm1_t, "mT1")
                    mT2 = ff_layer(mT1, m2_t, bm2_t, "mT2")

                    pqk = ps_mm.tile([P, D2], F32, tag="pm")
                    for k in range(2):
                        nc.tensor.matmul(pqk[:], lhsT=hTb[:, k, :],
                                         rhs=qk_t[:, k, :],
                                         start=(k == 0), stop=(k == 1))
                    oq = op.tile([P, D], BF16, tag="proj_q")
                    nc.vector.tensor_copy(oq[:], pqk[:, 0:D])
                    nc.sync.dma_start(q_dram[t * P:(t + 1) * P, :], oq[:])
                    ok_ = op.tile([P, D], F32, tag="proj_k")
                    nc.vector.tensor_copy(ok_[:], pqk[:, D:D2])
                    nc.sync.dma_start(kv_dram[t * P:(t + 1) * P, 0:D], ok_[:])
                    pv = ps_mm.tile([P, D2], F32, tag="pm")
                    for k in range(2):
                        nc.tensor.matmul(pv[:, 0:D], lhsT=mT2[:, k, :],
                                         rhs=v_t[:, k, :],
                                         start=(k == 0), stop=(k == 1))
                    ov = op.tile([P, D], F32, tag="proj_v")
                    nc.vector.tensor_copy(ov[:], pv[:, 0:D])
                    nc.sync.dma_start(kv_dram[t * P:(t + 1) * P, D:D2], ov[:])

            phase_a(hT_a_in, NT_A, q_a_sh, kv_a_sh)

            rg = [list(range(NC))]
            nc.gpsimd.collective_compute(
                "AllGather", ALU.bypass, replica_groups=rg,
                ins=[kv_a_sh.opt()], outs=[kv_a_full.opt()])
            nc.gpsimd.collective_compute(
                "AllGather", ALU.bypass, replica_groups=rg,
                ins=[q_a_sh.opt()], outs=[q_a_full.opt()])

            phase_a(hT_i_in, NT_I, q_inv, kv_inv)

            # ================= Phase B =================
            last_exp = [None]

            def phase_b(info, C, meta, src_sb, nw_sb, oet_dram, ote_dram,
                        q_tbl, kv_tbl, sink):
                numer = None
                qblk = None
                for g0 in range(0, C, SUBB):
                    n = min(SUBB, C - g0)
                    kvg = gp.tile([P, SUBB, D2], F32, tag="kvg")
                    nc.gpsimd.dma_gather(
                        out_ap=kvg[:, 0:n, :], in_ap=kv_tbl,
                        idxs_ap=src_sb[:, g0 * 8:(g0 + n) * 8],
                        num_idxs=n * P, num_idxs_reg=n * P, elem_size=D2)
                    qk = sp.tile([P, SUBB, D], F32, tag="qk")
                    ets = []
                    for j in range(n):
                        c = g0 + j
                        blk, first, last = info[c]
                        if first:
                            qblk = qbp.tile([P, D], BF16, tag="qblk")
                            nc.sync.dma_start(
                                qblk[:], q_tbl[blk * P:(blk + 1) * P, :])
                        ote = ohp.tile([P, P], BF16, tag="ote")
                        nc.sync.dma_start(ote[:], ote_dram[c, :, :])
                        oet = ohp.tile([P, P], BF16, tag="oet")
                        nc.sync.dma_start(oet[:], oet_dram[c, :, :])
                        ets.append(oet)
                        qe = ps_qe.tile([P, D], F32, tag="qe")
                        nc.tensor.matmul(qe[:], lhsT=ote[:], rhs=qblk[:],
                                         start=True, stop=True)
                        nc.vector.tensor_tensor(
                            qk[:, j, :], qe[:], kvg[:, j, 0:D], ALU.mult)
                    s4 = sp.tile([P, SUBB, H], F32, tag="s4")
                    nc.vector.tensor_reduce(
                        s4[:, 0:n, :],
                        qk[:, 0:n, :].rearrange("p c (h k) -> p c h k", h=H),
                        axis=mybir.AxisListType.X, op=ALU.add)
                    ex0 = sp.tile([P, SUBB, H], F32, tag="ex0")
                    last_exp[0] = nc.scalar.activation(
                        ex0[:, 0:n, :], s4[:, 0:n, :], AF.Exp, scale=0.125)
                    exb = sp.tile([P, SUBB, H], F32, tag="exb")
                    nc.vector.tensor_tensor(
                        exb[:, 0:n, :], ex0[:, 0:n, :], nw_sb[:, g0:g0 + n, :],
                        ALU.mult)
                    exe = sp.tile([P, SUBB, D + H], BF16, tag="exe")
                    nc.vector.tensor_tensor(
                        exe[:, 0:n, 0:D].rearrange("p c (h k) -> p c h k", h=H),
                        kvg[:, 0:n, D:D2].rearrange("p c (h k) -> p c h k", h=H),
                        exb[:, 0:n, :].unsqueeze(-1).to_broadcast((P, n, H, DK)),
                        ALU.mult)
                    nc.vector.tensor_copy(exe[:, 0:n, D:D + H], exb[:, 0:n, :])
                    for j in range(n):
                        c = g0 + j
                        blk, first, last = info[c]
                        if first:
                            numer = ps_nu.tile([P, D + H], F32, tag="nu")
                        nc.tensor.matmul(numer[:], lhsT=ets[j][:],
                                         rhs=exe[:, j, :],
                                         start=first, stop=last)
                        if last:
                            rows = slice(blk * P, (blk + 1) * P)
                            if sink[0] == "msg":
                                den = sp.tile([P, H], F32, tag="den")
                                nc.vector.tensor_scalar(
                                    den[:], numer[:, D:D + H], 1e-10, None,
                                    ALU.add)
                                rec = sp.tile([P, H], F32, tag="rec")
                                nc.vector.reciprocal(rec[:], den[:])
                                msg = op.tile([P, D], BF16, tag="msg")
                                nc.vector.tensor_tensor(
                                    msg[:].rearrange("p (h k) -> p h k", h=H),
                                    numer[:, 0:D].rearrange("p (h k) -> p h k", h=H),
                                    rec[:].unsqueeze(-1).to_broadcast((P, H, DK)),
                                    ALU.mult)
                                nc.sync.dma_start(sink[1][rows, :], msg[:])
                            else:
                                nu_sb = op.tile([P, D], F32, tag="nu_sb")
                                nc.vector.tensor_copy(nu_sb[:], numer[:, 0:D])
                                nc.sync.dma_start(sink[1][rows, :], nu_sb[:])
                                se_sb = sp.tile([P, H], F32, tag="se_sb")
                                nc.vector.tensor_copy(se_sb[:], numer[:, D:D + H])
                                nc.sync.dma_start(sink[2][rows, :], se_sb[:])
                for b, k in enumerate(meta):
                    if k != 0:
                        continue
                    rows = slice(b * P, (b + 1) * P)
                    if sink[0] == "msg":
                        nc.sync.dma_start(sink[1][rows, :], z256b[:])
                    else:
                        nc.sync.dma_start(sink[1][rows, :], z256[:])
                        nc.sync.dma_start(sink[2][rows, :], z4[:])

            d1_src_sb = wpool.tile([P, C1 * 8], I16, tag="d1_src_sb")
            nc.sync.dma_start(d1_src_sb[:], d1_src[:, :])
            d1_nw_sb = wpool.tile([P, C1, H], F32, tag="d1_nw_sb")
            nc.sync.dma_start(d1_nw_sb[:], d1_nw[:, :, :])
            d2_src_sb = wpool.tile([P, C2 * 8], I16, tag="d2_src_sb")
            nc.sync.dma_start(d2_src_sb[:], d2_src[:, :])
            d2_nw_sb = wpool.tile([P, C2, H], F32, tag="d2_nw_sb")
            nc.sync.dma_start(d2_nw_sb[:], d2_nw[:, :, :])

            phase_b(info1, C1, meta1, d1_src_sb, d1_nw_sb, d1_oet, d1_ote,
                    q_inv[:, :], kv_a_full[:, :], ("msg", msg_inv))
            phase_b(info2, C2, meta2, d2_src_sb, d2_nw_sb, d2_oet, d2_ote,
                    q_a_full[:, :], kv_inv[:, :], ("acc", numer_d, sumex_d))

            nc.gpsimd.collective_compute(
                "ReduceScatter", ALU.add, replica_groups=rg,
                ins=[numer_d.opt()], outs=[numer_sh.opt()])
            nc.gpsimd.collective_compute(
                "ReduceScatter", ALU.add, replica_groups=rg,
                ins=[sumex_d.opt()], outs=[sumex_sh.opt()])

            # ================= Phase C =================
            first_gelu = [None]

            def phase_c(ntiles, hT_in, msg_dram, out_dram):
                for t in range(ntiles):
                    cat = tp.tile([P, 4, P], BF16, tag="cat")
                    nc.sync.dma_start(cat[:, 0:2, :], hT_in[t * P:(t + 1) * P, :])
                    for k in range(2):
                        nc.sync.dma_start_transpose(
                            cat[:, 2 + k, :],
                            msg_dram[t * P:(t + 1) * P, k * P:(k + 1) * P])
                    y1 = tp.tile([P, 2, P], BF16, tag="y1")
                    for hf in range(2):
                        pm = ps_mm.tile([P, D2], F32, tag="pm")
                        for k in range(4):
                            nc.tensor.matmul(
                                pm[:, 0:P],
                                lhsT=u1_t[:, k, hf * P:(hf + 1) * P],
                                rhs=cat[:, k, :],
                                start=(k == 0), stop=(k == 3))
                        g = nc.scalar.activation(
                            y1[:, hf, :], pm[:, 0:P], AF.Gelu,
                            bias=bu1_t[:, hf:hf + 1])
                        if first_gelu[0] is None:
                            first_gelu[0] = g
                            if last_exp[0] is not None:
                                add_dep_helper(
                                    g.ins, last_exp[0].ins,
                                    reason="gelu after exp (ACT tables)")
                    po = ps_mm.tile([P, D2], F32, tag="pm")
                    for k in range(2):
                        nc.tensor.matmul(po[:, 0:D], lhsT=y1[:, k, :],
                                         rhs=u2_t[:, k, :],
                                         start=(k == 0), stop=False)
                    nc.tensor.matmul(po[:, 0:D], lhsT=ones_t[0:1, :],
                                     rhs=bu2_t[0:1, :], start=False, stop=True)
                    ot = op.tile([P, D], F32, tag="fin")
                    nc.scalar.activation(ot[:], po[:, 0:D], AF.Gelu)
                    nc.sync.dma_start(out_dram[t * P:(t + 1) * P, :], ot[:])

            phase_c(NT_I, hT_i_in, msg_inv, out_inv)

            for t in range(NT_A):
                nu = hp.tile([P, D], F32, tag="nu_f")
                nc.sync.dma_start(nu[:], numer_sh[t * P:(t + 1) * P, :])
                se = sp.tile([P, H], F32, tag="se_f")
                nc.sync.dma_start(se[:], sumex_sh[t * P:(t + 1) * P, :])
                den = sp.tile([P, H], F32, tag="den")
                nc.vector.tensor_scalar(den[:], se[:], 1e-10, None, ALU.add)
                rec = sp.tile([P, H], F32, tag="rec")
                nc.vector.reciprocal(rec[:], den[:])
                msg = op.tile([P, D], BF16, tag="msg")
                nc.vector.tensor_tensor(
                    msg[:].rearrange("p (h k) -> p h k", h=H),
                    nu[:].rearrange("p (h k) -> p h k", h=H),
                    rec[:].unsqueeze(-1).to_broadcast((P, H, DK)), ALU.mult)
                nc.sync.dma_start(msg_ast[t * P:(t + 1) * P, :], msg[:])

            phase_c(NT_A, hT_a_in, msg_ast, out_ast)

    nc.compile()
    return nc


# ----------------------------------------------------------------------------
# Entry point
# ----------------------------------------------------------------------------

def kernel(inv_h, asset_h, inv_norm_w, asset_norm_w,
           m_w1, m_b1, m_w2, m_b2, Wq, Wk, Wv,
           u_w1, u_b1, u_w2, u_b2, edge_tgt, edge_src):
    global _LAST_EXEC_NS
    bf = ml_dtypes.bfloat16
    inv_h = np.asarray(inv_h, np.float32)
    asset_h = np.asarray(asset_h, np.float32)
    inv_norm_w = np.asarray(inv_norm_w, np.float32)
    asset_norm_w = np.asarray(asset_norm_w, np.float32)
    edge_tgt = np.asarray(edge_tgt).astype(np.int64)
    edge_src = np.asarray(edge_src).astype(np.int64)
    m_w1, m_b1 = np.asarray(m_w1, np.float32), np.asarray(m_b1, np.float32)
    m_w2, m_b2 = np.asarray(m_w2, np.float32), np.asarray(m_b2, np.float32)
    Wq, Wk, Wv = (np.asarray(x, np.float32) for x in (Wq, Wk, Wv))
    u_w1, u_b1 = np.asarray(u_w1, np.float32), np.asarray(u_b1, np.float32)
    u_w2, u_b2 = np.asarray(u_w2, np.float32), np.asarray(u_b2, np.float32)

    ast_row = (edge_src // ASH) * ASHP + (edge_src % ASH)
    inv_core = edge_tgt // ISH
    inv_loc = edge_tgt - inv_core * ISH

    meta1, C1, d1 = _bucket(
        core=inv_core, blk=inv_loc // P, n_blocks=NT_I,
        srcidx=ast_row, t128_all=inv_loc % P, nw=inv_norm_w)
    meta2, C2, d2 = _bucket(
        core=inv_core, blk=ast_row // P, n_blocks=NB2,
        srcidx=inv_loc, t128_all=ast_row % P, nw=asset_norm_w)

    nc = _build(meta1, C1, meta2, C2)

    w_qk_h = np.concatenate([Wq.reshape(2, P, D), Wk.reshape(2, P, D)], axis=2)
    common = {
        "w_m1": m_w1.reshape(2, P, D).astype(bf),
        "w_m2": m_w2.reshape(2, P, D).astype(bf),
        "w_qk": w_qk_h.astype(bf),
        "w_v": Wv.reshape(2, P, D).astype(bf),
        "w_u1": u_w1.reshape(4, P, D).astype(bf),
        "w_u2": u_w2.reshape(2, P, D).astype(bf),
        "b_m1": m_b1.reshape(2, P).T.copy(),
        "b_m2": m_b2.reshape(2, P).T.copy(),
        "b_u1": u_b1.reshape(2, P).T.copy(),
        "b_u2r": u_b2.reshape(1, D).astype(bf),
    }

    in_maps = []
    for c in range(NC):
        s1, t1, n1 = d1[c]
        s2, t2, n2 = d2[c]
        m = dict(common)
        m["hT_i_in"] = _hT(_pad_rows(inv_h[c * ISH:(c + 1) * ISH], ISHP), NT_I)
        m["hT_a_in"] = _hT(_pad_rows(asset_h[c * ASH:(c + 1) * ASH], ASHP), NT_A)
        m["d1_src"] = _wrap16(s1, C1)
        m["d1_nw"] = _colmajor(n1, C1, rep=H)
        m["d1_oet"], m["d1_ote"] = _onehots(t1, C1)
        m["d2_src"] = _wrap16(s2, C2)
        m["d2_nw"] = _colmajor(n2, C2, rep=H)
        m["d2_oet"], m["d2_ote"] = _onehots(t2, C2)
        in_maps.append(m)

    res = bass_utils.run_bass_kernel_spmd(
        nc, in_maps, core_ids=list(range(NC)), trace=True)
    _LAST_EXEC_NS = res.exec_time_ns

    inv_out = np.concatenate(
        [res.results[c]["out_inv"][:ISH] for c in range(NC)], axis=0)
    ast_out = np.concatenate(
        [res.results[c]["out_ast"][:ASH] for c in range(NC)], axis=0)
    return inv_out, ast_out


# revision 11
# speedup vs baseline: 1.3291x; 1.0757x over previous
"""Trainium2 Bass kernel for nn_MessagePassingLayer (bipartite GNN attention
message passing), distributed over 8 NeuronCores.

v5 design notes:
  - Node tables row-sharded 8 ways (inv 6250->6272 padded, asset 1250->1280).
  - dma_gather on TRN2 is descriptor-generation bound (~5.9ns/idx + 2.6us/call,
    independent of row bytes), so K|V are packed into one fp32 [N,512] row
    (one gather per edge total) and Q is never gathered: per target block the
    128 Q rows are densely loaded and Qe is formed on the PE with a
    host-precomputed transposed one-hot (exact in bf16).
  - Phase A: project local shard; FF runs feature-major off a host-supplied
    transposed bf16 copy of h, so there are no on-device transposes.  Small
    asset tables (kv_a fp32, q_a bf16) are AllGathered; the 51MB investor
    tables stay local (dir2 is shard-by-source + ReduceScatter of the segment
    stats instead).
  - Phase B per 128-edge chunk: gather KV[src]; Qe = oh_te @ Qblk (PE);
    qk = Qe*Ke (DVE, fp32); per-head reduce; ex = nw*exp(s/8) (max-term
    dropped -- exact to 1e-10 rel here: ex equals the reference's exactly
    when the segment max < 0, and the 1e-10 eps is negligible otherwise);
    exe=[ex*V | ex] in bf16; numer|sumex accumulated per target block by one
    one-hot matmul into fp32 PSUM; divide at block end.  Batched DVE ops
    amortize op overhead across 8-chunk gather batches.
  - dir2 partial numer/sumex (over all 10240 asset rows) are ReduceScattered.
  - Phase C: out = FF([h, msg]); msg transposed via hardware DMA-transpose
    (bf16); matmuls bf16, final gelu fp32 out.
  - gelu and exp live in different ACT table sets; an explicit dep keeps all
    phase-C gelus after the last phase-B exp to avoid table-reload thrash.
"""

import ml_dtypes
import numpy as np
from contextlib import ExitStack

import concourse.bass as bass
import concourse.tile as tile
from concourse.tile import add_dep_helper
from concourse import bacc, mybir
from concourse import bass_utils

F32 = mybir.dt.float32
BF16 = mybir.dt.bfloat16
I16 = mybir.dt.int16
AF = mybir.ActivationFunctionType
ALU = mybir.AluOpType

I_N, A_N, E_N = 50000, 10000, 200000
D, H, DK = 256, 4, 64
D2 = 2 * D
NC = 8
P = 128
ISH, ASH = I_N // NC, A_N // NC            # 6250, 1250
ISHP, ASHP = 6272, 1280
NT_I, NT_A = ISHP // P, ASHP // P          # 49, 10
NB2 = ASHP * NC // P                       # 80 global asset blocks
SUBB = 8                                   # chunks per gather call (1024 idx cap)

_LAST_EXEC_NS = None


# ----------------------------------------------------------------------------
# Host-side edge preparation
# ----------------------------------------------------------------------------

def _bucket(core, blk, n_blocks, srcidx, t128_all, nw):
    buckets = {}
    for c in range(NC):
        m_c = core == c
        for b in range(n_blocks):
            buckets[(c, b)] = np.nonzero(m_c & (blk == b))[0]
    meta = [max((len(buckets[(c, b)]) + P - 1) // P for c in range(NC))
            for b in range(n_blocks)]
    C = sum(meta)
    cores_out = []
    for c in range(NC):
        s16 = np.zeros(C * P, np.int64)
        t128 = np.full(C * P, -1, np.int64)
        nwv = np.zeros(C * P, np.float32)
        pos = 0
        for b in range(n_blocks):
            idx = buckets[(c, b)]
            n = len(idx)
            if meta[b] == 0:
                continue
            sl = slice(pos, pos + n)
            s16[sl] = srcidx[idx]
            t128[sl] = t128_all[idx]
            nwv[sl] = nw[idx]
            pos += meta[b] * P
        assert pos == C * P
        cores_out.append((s16, t128, nwv))
    return meta, C, cores_out


def _wrap16(flat_idx, C):
    assert flat_idx.max(initial=0) < 32768 and flat_idx.min(initial=0) >= 0
    w = flat_idx.astype(np.int16).reshape(C * 8, 16).T
    return np.tile(w, (8, 1)).copy()


def _colmajor(flat, C, rep=1):
    a = flat.reshape(C, P).T.copy()
    if rep == 1:
        return a
    return np.repeat(a[:, :, None], rep, axis=2).copy()


def _onehots(t128, C):
    """[C*128] targets (-1=pad) -> combined [C, 128, 256] = [oh_te | oh_et]."""
    bf = ml_dtypes.bfloat16
    t = t128.reshape(C, P, 1)
    oh_et = (t == np.arange(P).reshape(1, 1, P)).astype(bf)
    out = np.empty((C, P, 2 * P), bf)
    out[:, :, 0:P] = oh_et.transpose(0, 2, 1)
    out[:, :, P:2 * P] = oh_et
    return out


def _pad_rows(a, n):
    out = np.zeros((n, a.shape[1]), a.dtype)
    out[: a.shape[0]] = a
    return out


def _hT(h_pad, ntiles):
    """[N,256] f32 -> bf16 transposed-tile layout [N,256]:
    row (t*128+f), col (k*128+n) = h[t*128+n, k*128+f]."""
    bf = ml_dtypes.bfloat16
    x = h_pad.reshape(ntiles, P, 2, P)          # (t, n, k, f)
    x = x.transpose(0, 3, 2, 1)                 # (t, f, k, n)
    return np.ascontiguousarray(x.reshape(ntiles * P, D).astype(bf))


def _chunk_info(meta):
    info = []
    for b, k in enumerate(meta):
        for j in range(k):
            info.append((b, j == 0, j == k - 1))
    return info


# ----------------------------------------------------------------------------
# Device program
# ----------------------------------------------------------------------------

def _build(meta1, C1, meta2, C2):
    nc = bacc.Bacc("TRN2", target_bir_lowering=False, debug=False,
                   enable_asserts=True, num_devices=NC)

    hT_i_in = nc.dram_tensor("hT_i_in", [NT_I * P, D], BF16, kind="ExternalInput")
    hT_a_in = nc.dram_tensor("hT_a_in", [NT_A * P, D], BF16, kind="ExternalInput")
    w_m1 = nc.dram_tensor("w_m1", [2, P, D], BF16, kind="ExternalInput")
    w_m2 = nc.dram_tensor("w_m2", [2, P, D], BF16, kind="ExternalInput")
    w_qk = nc.dram_tensor("w_qk", [2, P, D2], BF16, kind="ExternalInput")
    w_v = nc.dram_tensor("w_v", [2, P, D], BF16, kind="ExternalInput")
    w_u1 = nc.dram_tensor("w_u1", [4, P, D], BF16, kind="ExternalInput")
    w_u2 = nc.dram_tensor("w_u2", [2, P, D], BF16, kind="ExternalInput")
    b_m1 = nc.dram_tensor("b_m1", [P, 2], F32, kind="ExternalInput")
    b_m2 = nc.dram_tensor("b_m2", [P, 2], F32, kind="ExternalInput")
    b_u1 = nc.dram_tensor("b_u1", [P, 2], F32, kind="ExternalInput")
    b_u2r = nc.dram_tensor("b_u2r", [1, D], BF16, kind="ExternalInput")

    d1_src = nc.dram_tensor("d1_src", [P, C1 * 8], I16, kind="ExternalInput")
    d1_nw = nc.dram_tensor("d1_nw", [P, C1, H], F32, kind="ExternalInput")
    d1_oh = nc.dram_tensor("d1_oh", [C1, P, 2 * P], BF16, kind="ExternalInput")
    d2_src = nc.dram_tensor("d2_src", [P, C2 * 8], I16, kind="ExternalInput")
    d2_nw = nc.dram_tensor("d2_nw", [P, C2, H], F32, kind="ExternalInput")
    d2_oh = nc.dram_tensor("d2_oh", [C2, P, 2 * P], BF16, kind="ExternalInput")

    out_inv = nc.dram_tensor("out_inv", [ISHP, D], F32, kind="ExternalOutput")
    out_ast = nc.dram_tensor("out_ast", [ASHP, D], F32, kind="ExternalOutput")

    info1 = _chunk_info(meta1)
    info2 = _chunk_info(meta2)

    with tile.TileContext(nc) as tc:
        with ExitStack() as ctx:
            wpool = ctx.enter_context(tc.tile_pool(name="w", bufs=1))
            hp = ctx.enter_context(tc.tile_pool(name="hp", bufs=3))
            tp = ctx.enter_context(tc.tile_pool(name="tp", bufs=3))
            op = ctx.enter_context(tc.tile_pool(name="op", bufs=3))
            gp = ctx.enter_context(tc.tile_pool(name="gp", bufs=2))
            sp = ctx.enter_context(tc.tile_pool(name="sp", bufs=2))
            ohp = ctx.enter_context(tc.tile_pool(name="ohp", bufs=4))
            qbp = ctx.enter_context(tc.tile_pool(name="qbp", bufs=2))
            ps_mm = ctx.enter_context(tc.tile_pool(name="ps_mm", bufs=2, space="PSUM"))
            ps_nu = ctx.enter_context(tc.tile_pool(name="ps_nu", bufs=2, space="PSUM"))
            ps_qe = ctx.enter_context(tc.tile_pool(name="ps_qe", bufs=3, space="PSUM"))
            dram = ctx.enter_context(tc.tile_pool(name="dram", bufs=1, space="DRAM"))

            ones_t = wpool.tile([1, P], BF16, tag="ones_t")
            nc.vector.memset(ones_t[:], 1.0)
            z256 = wpool.tile([P, D], F32, tag="z256")
            nc.vector.memset(z256[:], 0.0)
            z4 = wpool.tile([P, H], F32, tag="z4")
            nc.vector.memset(z4[:], 0.0)
            z256b = wpool.tile([P, D], BF16, tag="z256b")
            nc.vector.memset(z256b[:], 0.0)

            def load_w(dram_w, nk, nd, dt, tag):
                t = wpool.tile([P, nk, nd], dt, tag=tag)
                for k in range(nk):
                    nc.sync.dma_start(t[:, k, :], dram_w[k, :, :])
                return t

            m1_t = load_w(w_m1, 2, D, BF16, "wm1")
            m2_t = load_w(w_m2, 2, D, BF16, "wm2")
            qk_t = load_w(w_qk, 2, D2, BF16, "wqk")
            v_t = load_w(w_v, 2, D, BF16, "wv")
            u1_t = load_w(w_u1, 4, D, BF16, "wu1")
            u2_t = load_w(w_u2, 2, D, BF16, "wu2")

            def load_b(dram_b, tag):
                t = wpool.tile([P, 2], F32, tag=tag)
                nc.sync.dma_start(t[:], dram_b[:, :])
                return t

            bm1_t, bm2_t, bu1_t = load_b(b_m1, "bm1"), load_b(b_m2, "bm2"), load_b(b_u1, "bu1")
            bu2_t = wpool.tile([1, D], BF16, tag="bu2")
            nc.sync.dma_start(bu2_t[:], b_u2r[:, :])

            q_inv = dram.tile([ISHP, D], BF16, tag="q_inv")
            kv_inv = dram.tile([ISHP, D2], F32, tag="kv_inv")
            q_a_sh = dram.tile([ASHP, D], BF16, tag="q_a_sh")
            kv_a_sh = dram.tile([ASHP, D2], F32, tag="kv_a_sh")
            q_a_full = dram.tile([ASHP * NC, D], BF16, tag="q_a_full")
            kv_a_full = dram.tile([ASHP * NC, D2], F32, tag="kv_a_full")
            numer_d = dram.tile([ASHP * NC, D], F32, tag="numer_d")
            sumex_d = dram.tile([ASHP * NC, H], F32, tag="sumex_d")
            numer_sh = dram.tile([ASHP, D], F32, tag="numer_sh")
            sumex_sh = dram.tile([ASHP, H], F32, tag="sumex_sh")
            msg_inv = dram.tile([ISHP, D], BF16, tag="msg_inv")
            msg_ast = dram.tile([ASHP, D], BF16, tag="msg_ast")

            # ================= Phase A =================
            def phase_a(hT_in, ntiles, q_dram, kv_dram):
                for t in range(ntiles):
                    hTb = tp.tile([P, 2, P], BF16, tag="hTb")
                    nc.sync.dma_start(hTb[:], hT_in[t * P:(t + 1) * P, :])

                    def ff_layer(inT, w_tile, b_tile, tag):
                        outT = tp.tile([P, 2, P], BF16, tag=tag)
                        for hf in range(2):
                            pm = ps_mm.tile([P, D2], F32, tag="pm")
                            for k in range(2):
                                nc.tensor.matmul(
                                    pm[:, 0:P],
                                    lhsT=w_tile[:, k, hf * P:(hf + 1) * P],
                                    rhs=inT[:, k, :],
                                    start=(k == 0), stop=(k == 1))
                            nc.scalar.activation(
                                outT[:, hf, :], pm[:, 0:P], AF.Gelu,
                                bias=b_tile[:, hf:hf + 1])
                        return outT

                    mT1 = ff_layer(hTb, m1_t, bm1_t, "mT1")
                    mT2 = ff_layer(mT1, m2_t, bm2_t, "mT2")

                    pqk = ps_mm.tile([P, D2], F32, tag="pm")
                    for k in range(2):
                        nc.tensor.matmul(pqk[:], lhsT=hTb[:, k, :],
                                         rhs=qk_t[:, k, :],
                                         start=(k == 0), stop=(k == 1))
                    oq = op.tile([P, D], BF16, tag="proj_q")
                    nc.vector.tensor_copy(oq[:], pqk[:, 0:D])
                    nc.scalar.dma_start(q_dram[t * P:(t + 1) * P, :], oq[:])
                    ok_ = op.tile([P, D], F32, tag="proj_k")
                    nc.vector.tensor_copy(ok_[:], pqk[:, D:D2])
                    nc.scalar.dma_start(kv_dram[t * P:(t + 1) * P, 0:D], ok_[:])
                    pv = ps_mm.tile([P, D2], F32, tag="pm")
                    for k in range(2):
                        nc.tensor.matmul(pv[:, 0:D], lhsT=mT2[:, k, :],
                                         rhs=v_t[:, k, :],
                                         start=(k == 0), stop=(k == 1))
                    ov = op.tile([P, D], F32, tag="proj_v")
                    nc.vector.tensor_copy(ov[:], pv[:, 0:D])
                    nc.scalar.dma_start(kv_dram[t * P:(t + 1) * P, D:D2], ov[:])

            phase_a(hT_a_in, NT_A, q_a_sh, kv_a_sh)

            rg = [list(range(NC))]
            nc.gpsimd.collective_compute(
                "AllGather", ALU.bypass, replica_groups=rg,
                ins=[kv_a_sh.opt()], outs=[kv_a_full.opt()])
            nc.gpsimd.collective_compute(
                "AllGather", ALU.bypass, replica_groups=rg,
                ins=[q_a_sh.opt()], outs=[q_a_full.opt()])

            phase_a(hT_i_in, NT_I, q_inv, kv_inv)

            # ================= Phase B =================
            last_exp = [None]

            def phase_b(info, C, meta, src_sb, nw_sb, oh_dram,
                        q_tbl, kv_tbl, sink):
                numer = None
                qblk = None
                for g0 in range(0, C, SUBB):
                    n = min(SUBB, C - g0)
                    kvg = gp.tile([P, SUBB, D2], F32, tag="kvg")
                    nc.gpsimd.dma_gather(
                        out_ap=kvg[:, 0:n, :], in_ap=kv_tbl,
                        idxs_ap=src_sb[:, g0 * 8:(g0 + n) * 8],
                        num_idxs=n * P, num_idxs_reg=n * P, elem_size=D2)
                    ohg = ohp.tile([P, SUBB, 2 * P], BF16, tag="ohg")
                    nc.sync.dma_start(
                        ohg[:, 0:n, :],
                        oh_dram[g0:g0 + n].transpose([1, 0, 2]))
                    qk = sp.tile([P, SUBB, D], F32, tag="qk")
                    for j in range(n):
                        c = g0 + j
                        blk, first, last = info[c]
                        if first:
                            qblk = qbp.tile([P, D], BF16, tag="qblk")
                            nc.sync.dma_start(
                                qblk[:], q_tbl[blk * P:(blk + 1) * P, :])
                        qe = ps_qe.tile([P, D], F32, tag="qe")
                        nc.tensor.matmul(qe[:], lhsT=ohg[:, j, 0:P], rhs=qblk[:],
                                         start=True, stop=True)
                        nc.vector.tensor_tensor(
                            qk[:, j, :], qe[:], kvg[:, j, 0:D], ALU.mult)
                    s4 = sp.tile([P, SUBB, H], F32, tag="s4")
                    nc.vector.tensor_reduce(
                        s4[:, 0:n, :],
                        qk[:, 0:n, :].rearrange("p c (h k) -> p c h k", h=H),
                        axis=mybir.AxisListType.X, op=ALU.add)
                    ex0 = sp.tile([P, SUBB, H], F32, tag="ex0")
                    last_exp[0] = nc.scalar.activation(
                        ex0[:, 0:n, :], s4[:, 0:n, :], AF.Exp, scale=0.125)
                    exb = sp.tile([P, SUBB, H], F32, tag="exb")
                    nc.vector.tensor_tensor(
                        exb[:, 0:n, :], ex0[:, 0:n, :], nw_sb[:, g0:g0 + n, :],
                        ALU.mult)
                    exe = sp.tile([P, SUBB, D + H], BF16, tag="exe")
                    nc.vector.tensor_tensor(
                        exe[:, 0:n, 0:D].rearrange("p c (h k) -> p c h k", h=H),
                        kvg[:, 0:n, D:D2].rearrange("p c (h k) -> p c h k", h=H),
                        exb[:, 0:n, :].unsqueeze(-1).to_broadcast((P, n, H, DK)),
                        ALU.mult)
                    nc.vector.tensor_copy(exe[:, 0:n, D:D + H], exb[:, 0:n, :])
                    for j in range(n):
                        c = g0 + j
                        blk, first, last = info[c]
                        if first:
                            numer = ps_nu.tile([P, D + H], F32, tag="nu")
                        nc.tensor.matmul(numer[:], lhsT=ohg[:, j, P:2 * P],
                                         rhs=exe[:, j, :],
                                         start=first, stop=last)
                        if last:
                            rows = slice(blk * P, (blk + 1) * P)
                            if sink[0] == "msg":
                                den = sp.tile([P, H], F32, tag="den")
                                nc.vector.tensor_scalar(
                                    den[:], numer[:, D:D + H], 1e-10, None,
                                    ALU.add)
                                rec = sp.tile([P, H], F32, tag="rec")
                                nc.vector.reciprocal(rec[:], den[:])
                                msg = op.tile([P, D], BF16, tag="msg")
                                nc.vector.tensor_tensor(
                                    msg[:].rearrange("p (h k) -> p h k", h=H),
                                    numer[:, 0:D].rearrange("p (h k) -> p h k", h=H),
                                    rec[:].unsqueeze(-1).to_broadcast((P, H, DK)),
                                    ALU.mult)
                                nc.scalar.dma_start(sink[1][rows, :], msg[:])
                            else:
                                nu_sb = op.tile([P, D], F32, tag="nu_sb")
                                nc.vector.tensor_copy(nu_sb[:], numer[:, 0:D])
                                nc.scalar.dma_start(sink[1][rows, :], nu_sb[:])
                                se_sb = sp.tile([P, H], F32, tag="se_sb")
                                nc.vector.tensor_copy(se_sb[:], numer[:, D:D + H])
                                nc.scalar.dma_start(sink[2][rows, :], se_sb[:])
                for b, k in enumerate(meta):
                    if k != 0:
                        continue
                    rows = slice(b * P, (b + 1) * P)
                    if sink[0] == "msg":
                        nc.sync.dma_start(sink[1][rows, :], z256b[:])
                    else:
                        nc.sync.dma_start(sink[1][rows, :], z256[:])
                        nc.sync.dma_start(sink[2][rows, :], z4[:])

            d1_src_sb = wpool.tile([P, C1 * 8], I16, tag="d1_src_sb")
            nc.sync.dma_start(d1_src_sb[:], d1_src[:, :])
            d1_nw_sb = wpool.tile([P, C1, H], F32, tag="d1_nw_sb")
            nc.sync.dma_start(d1_nw_sb[:], d1_nw[:, :, :])
            d2_src_sb = wpool.tile([P, C2 * 8], I16, tag="d2_src_sb")
            nc.sync.dma_start(d2_src_sb[:], d2_src[:, :])
            d2_nw_sb = wpool.tile([P, C2, H], F32, tag="d2_nw_sb")
            nc.sync.dma_start(d2_nw_sb[:], d2_nw[:, :, :])

            phase_b(info1, C1, meta1, d1_src_sb, d1_nw_sb, d1_oh,
                    q_inv[:, :], kv_a_full[:, :], ("msg", msg_inv))
            phase_b(info2, C2, meta2, d2_src_sb, d2_nw_sb, d2_oh,
                    q_a_full[:, :], kv_inv[:, :], ("acc", numer_d, sumex_d))

            nc.gpsimd.collective_compute(
                "ReduceScatter", ALU.add, replica_groups=rg,
                ins=[numer_d.opt()], outs=[numer_sh.opt()])
            nc.gpsimd.collective_compute(
                "ReduceScatter", ALU.add, replica_groups=rg,
                ins=[sumex_d.opt()], outs=[sumex_sh.opt()])

            # ================= Phase C =================
            first_gelu = [None]

            def phase_c(ntiles, hT_in, msg_dram, out_dram):
                for t in range(ntiles):
                    cat = tp.tile([P, 4, P], BF16, tag="cat")
                    nc.sync.dma_start(cat[:, 0:2, :], hT_in[t * P:(t + 1) * P, :])
                    for k in range(2):
                        nc.sync.dma_start_transpose(
                            cat[:, 2 + k, :],
                            msg_dram[t * P:(t + 1) * P, k * P:(k + 1) * P])
                    y1 = tp.tile([P, 2, P], BF16, tag="y1")
                    for hf in range(2):
                        pm = ps_mm.tile([P, D2], F32, tag="pm")
                        for k in range(4):
                            nc.tensor.matmul(
                                pm[:, 0:P],
                                lhsT=u1_t[:, k, hf * P:(hf + 1) * P],
                                rhs=cat[:, k, :],
                                start=(k == 0), stop=(k == 3))
                        g = nc.scalar.activation(
                            y1[:, hf, :], pm[:, 0:P], AF.Gelu,
                            bias=bu1_t[:, hf:hf + 1])
                        if first_gelu[0] is None:
                            first_gelu[0] = g
                            if last_exp[0] is not None:
                                add_dep_helper(
                                    g.ins, last_exp[0].ins,
                                    reason="gelu after exp (ACT tables)")
                    po = ps_mm.tile([P, D2], F32, tag="pm")
                    for k in range(2):
                        nc.tensor.matmul(po[:, 0:D], lhsT=y1[:, k, :],
                                         rhs=u2_t[:, k, :],
                                         start=(k == 0), stop=False)
                    nc.tensor.matmul(po[:, 0:D], lhsT=ones_t[0:1, :],
                                     rhs=bu2_t[0:1, :], start=False, stop=True)
                    ot = op.tile([P, D], F32, tag="fin")
                    nc.scalar.activation(ot[:], po[:, 0:D], AF.Gelu)
                    nc.scalar.dma_start(out_dram[t * P:(t + 1) * P, :], ot[:])

            phase_c(NT_I, hT_i_in, msg_inv, out_inv)

            for t in range(NT_A):
                nu = hp.tile([P, D], F32, tag="nu_f")
                nc.sync.dma_start(nu[:], numer_sh[t * P:(t + 1) * P, :])
                se = sp.tile([P, H], F32, tag="se_f")
                nc.sync.dma_start(se[:], sumex_sh[t * P:(t + 1) * P, :])
                den = sp.tile([P, H], F32, tag="den")
                nc.vector.tensor_scalar(den[:], se[:], 1e-10, None, ALU.add)
                rec = sp.tile([P, H], F32, tag="rec")
                nc.vector.reciprocal(rec[:], den[:])
                msg = op.tile([P, D], BF16, tag="msg")
                nc.vector.tensor_tensor(
                    msg[:].rearrange("p (h k) -> p h k", h=H),
                    nu[:].rearrange("p (h k) -> p h k", h=H),
                    rec[:].unsqueeze(-1).to_broadcast((P, H, DK)), ALU.mult)
                nc.sync.dma_start(msg_ast[t * P:(t + 1) * P, :], msg[:])

            phase_c(NT_A, hT_a_in, msg_ast, out_ast)

    nc.compile()
    return nc


# ----------------------------------------------------------------------------
# Entry point
# ----------------------------------------------------------------------------

def kernel(inv_h, asset_h, inv_norm_w, asset_norm_w,
           m_w1, m_b1, m_w2, m_b2, Wq, Wk, Wv,
           u_w1, u_b1, u_w2, u_b2, edge_tgt, edge_src):
    global _LAST_EXEC_NS
    bf = ml_dtypes.bfloat16
    inv_h = np.asarray(inv_h, np.float32)
    asset_h = np.asarray(asset_h, np.float32)
    inv_norm_w = np.asarray(inv_norm_w, np.float32)
    asset_norm_w = np.asarray(asset_norm_w, np.float32)
    edge_tgt = np.asarray(edge_tgt).astype(np.int64)
    edge_src = np.asarray(edge_src).astype(np.int64)
    m_w1, m_b1 = np.asarray(m_w1, np.float32), np.asarray(m_b1, np.float32)
    m_w2, m_b2 = np.asarray(m_w2, np.float32), np.asarray(m_b2, np.float32)
    Wq, Wk, Wv = (np.asarray(x, np.float32) for x in (Wq, Wk, Wv))
    u_w1, u_b1 = np.asarray(u_w1, np.float32), np.asarray(u_b1, np.float32)
    u_w2, u_b2 = np.asarray(u_w2, np.float32), np.asarray(u_b2, np.float32)

    ast_row = (edge_src // ASH) * ASHP + (edge_src % ASH)
    inv_core = edge_tgt // ISH
    inv_loc = edge_tgt - inv_core * ISH

    meta1, C1, d1 = _bucket(
        core=inv_core, blk=inv_loc // P, n_blocks=NT_I,
        srcidx=ast_row, t128_all=inv_loc % P, nw=inv_norm_w)
    meta2, C2, d2 = _bucket(
        core=inv_core, blk=ast_row // P, n_blocks=NB2,
        srcidx=inv_loc, t128_all=ast_row % P, nw=asset_norm_w)

    nc = _build(meta1, C1, meta2, C2)

    w_qk_h = np.concatenate([Wq.reshape(2, P, D), Wk.reshape(2, P, D)], axis=2)
    common = {
        "w_m1": m_w1.reshape(2, P, D).astype(bf),
        "w_m2": m_w2.reshape(2, P, D).astype(bf),
        "w_qk": w_qk_h.astype(bf),
        "w_v": Wv.reshape(2, P, D).astype(bf),
        "w_u1": u_w1.reshape(4, P, D).astype(bf),
        "w_u2": u_w2.reshape(2, P, D).astype(bf),
        "b_m1": m_b1.reshape(2, P).T.copy(),
        "b_m2": m_b2.reshape(2, P).T.copy(),
        "b_u1": u_b1.reshape(2, P).T.copy(),
        "b_u2r": u_b2.reshape(1, D).astype(bf),
    }

    in_maps = []
    for c in range(NC):
        s1, t1, n1 = d1[c]
        s2, t2, n2 = d2[c]
        m = dict(common)
        m["hT_i_in"] = _hT(_pad_rows(inv_h[c * ISH:(c + 1) * ISH], ISHP), NT_I)
        m["hT_a_in"] = _hT(_pad_rows(asset_h[c * ASH:(c + 1) * ASH], ASHP), NT_A)
        m["d1_src"] = _wrap16(s1, C1)
        m["d1_nw"] = _colmajor(n1, C1, rep=H)
        m["d1_oh"] = _onehots(t1, C1)
        m["d2_src"] = _wrap16(s2, C2)
        m["d2_nw"] = _colmajor(n2, C2, rep=H)
        m["d2_oh"] = _onehots(t2, C2)
        in_maps.append(m)

    res = bass_utils.run_bass_kernel_spmd(
        nc, in_maps, core_ids=list(range(NC)), trace=True)
    _LAST_EXEC_NS = res.exec_time_ns

    inv_out = np.concatenate(
        [res.results[c]["out_inv"][:ISH] for c in range(NC)], axis=0)
    ast_out = np.concatenate(
        [res.results[c]["out_ast"][:ASH] for c in range(NC)], axis=0)
    return inv_out, ast_out


# revision 13
# speedup vs baseline: 1.5517x; 1.1675x over previous
"""Trainium2 Bass kernel for nn_MessagePassingLayer (bipartite GNN attention
message passing), distributed over 8 NeuronCores.

v5 design notes:
  - Node tables row-sharded 8 ways (inv 6250->6272 padded, asset 1250->1280).
  - dma_gather on TRN2 is descriptor-generation bound (~5.9ns/idx + 2.6us/call,
    independent of row bytes), so K|V are packed into one fp32 [N,512] row
    (one gather per edge total) and Q is never gathered: per target block the
    128 Q rows are densely loaded and Qe is formed on the PE with a
    host-precomputed transposed one-hot (exact in bf16).
  - Phase A: project local shard; FF runs feature-major off a host-supplied
    transposed bf16 copy of h, so there are no on-device transposes.  Small
    asset tables (kv_a fp32, q_a bf16) are AllGathered; the 51MB investor
    tables stay local (dir2 is shard-by-source + ReduceScatter of the segment
    stats instead).
  - Phase B per 128-edge chunk: gather KV[src]; Qe = oh_te @ Qblk (PE);
    qk = Qe*Ke (DVE, fp32); per-head reduce; ex = nw*exp(s/8) (max-term
    dropped -- exact to 1e-10 rel here: ex equals the reference's exactly
    when the segment max < 0, and the 1e-10 eps is negligible otherwise);
    exe=[ex*V | ex] in bf16; numer|sumex accumulated per target block by one
    one-hot matmul into fp32 PSUM; divide at block end.  Batched DVE ops
    amortize op overhead across 8-chunk gather batches.
  - dir2 partial numer/sumex (over all 10240 asset rows) are ReduceScattered.
  - Phase C: out = FF([h, msg]); msg transposed via hardware DMA-transpose
    (bf16); matmuls bf16, final gelu fp32 out.
  - gelu and exp live in different ACT table sets; an explicit dep keeps all
    phase-C gelus after the last phase-B exp to avoid table-reload thrash.
"""

import ml_dtypes
import numpy as np
from contextlib import ExitStack

import concourse.bass as bass
import concourse.tile as tile
from concourse.tile import add_dep_helper
from concourse import bacc, mybir
from concourse import bass_utils

F32 = mybir.dt.float32
BF16 = mybir.dt.bfloat16
I16 = mybir.dt.int16
AF = mybir.ActivationFunctionType
ALU = mybir.AluOpType

I_N, A_N, E_N = 50000, 10000, 200000
D, H, DK = 256, 4, 64
D2 = 2 * D
NC = 8
P = 128
ISH, ASH = I_N // NC, A_N // NC            # 6250, 1250
ISHP, ASHP = 6272, 1280
NT_I, NT_A = ISHP // P, ASHP // P          # 49, 10
NB2 = ASHP * NC // P                       # 80 global asset blocks
SUBB = 8                                   # chunks per gather call (1024 idx cap)

_LAST_EXEC_NS = None


# ----------------------------------------------------------------------------
# Host-side edge preparation
# ----------------------------------------------------------------------------

def _bucket(core, blk, n_blocks, srcidx, t128_all, nw):
    buckets = {}
    for c in range(NC):
        m_c = core == c
        for b in range(n_blocks):
            buckets[(c, b)] = np.nonzero(m_c & (blk == b))[0]
    meta = [max((len(buckets[(c, b)]) + P - 1) // P for c in range(NC))
            for b in range(n_blocks)]
    C = sum(meta)
    cores_out = []
    for c in range(NC):
        s16 = np.zeros(C * P, np.int64)
        t128 = np.full(C * P, -1, np.int64)
        nwv = np.zeros(C * P, np.float32)
        pos = 0
        for b in range(n_blocks):
            idx = buckets[(c, b)]
            n = len(idx)
            if meta[b] == 0:
                continue
            sl = slice(pos, pos + n)
            s16[sl] = srcidx[idx]
            t128[sl] = t128_all[idx]
            nwv[sl] = nw[idx]
            pos += meta[b] * P
        assert pos == C * P
        cores_out.append((s16, t128, nwv))
    return meta, C, cores_out


def _wrap16(flat_idx, C):
    assert flat_idx.max(initial=0) < 32768 and flat_idx.min(initial=0) >= 0
    w = flat_idx.astype(np.int16).reshape(C * 8, 16).T
    return np.tile(w, (8, 1)).copy()


def _colmajor(flat, C, rep=1):
    a = flat.reshape(C, P).T.copy()
    if rep == 1:
        return a
    return np.repeat(a[:, :, None], rep, axis=2).copy()


def _onehots(t128, C):
    """[C*128] targets (-1=pad) -> combined [C, 128, 256] = [oh_te | oh_et]."""
    bf = ml_dtypes.bfloat16
    t = t128.reshape(C, P, 1)
    oh_et = (t == np.arange(P).reshape(1, 1, P)).astype(bf)
    out = np.empty((C, P, 2 * P), bf)
    out[:, :, 0:P] = oh_et.transpose(0, 2, 1)
    out[:, :, P:2 * P] = oh_et
    return out


def _pad_rows(a, n):
    out = np.zeros((n, a.shape[1]), a.dtype)
    out[: a.shape[0]] = a
    return out


def _hT(h_pad, ntiles):
    """[N,256] f32 -> bf16 transposed-tile layout [N,256]:
    row (t*128+f), col (k*128+n) = h[t*128+n, k*128+f]."""
    bf = ml_dtypes.bfloat16
    x = h_pad.reshape(ntiles, P, 2, P)          # (t, n, k, f)
    x = x.transpose(0, 3, 2, 1)                 # (t, f, k, n)
    return np.ascontiguousarray(x.reshape(ntiles * P, D).astype(bf))


def _chunk_info(meta):
    info = []
    for b, k in enumerate(meta):
        for j in range(k):
            info.append((b, j == 0, j == k - 1))
    return info


# ----------------------------------------------------------------------------
# Device program
# ----------------------------------------------------------------------------

def _build(meta1, C1, meta2, C2):
    nc = bacc.Bacc("TRN2", target_bir_lowering=False, debug=False,
                   enable_asserts=True, num_devices=NC)

    hT_i_in = nc.dram_tensor("hT_i_in", [NT_I * P, D], BF16, kind="ExternalInput")
    hT_a_in = nc.dram_tensor("hT_a_in", [NT_A * P, D], BF16, kind="ExternalInput")
    w_m1 = nc.dram_tensor("w_m1", [2, P, D], BF16, kind="ExternalInput")
    w_m2 = nc.dram_tensor("w_m2", [2, P, D], BF16, kind="ExternalInput")
    w_qk = nc.dram_tensor("w_qk", [2, P, D2], BF16, kind="ExternalInput")
    w_v = nc.dram_tensor("w_v", [2, P, D], BF16, kind="ExternalInput")
    w_u1 = nc.dram_tensor("w_u1", [4, P, D], BF16, kind="ExternalInput")
    w_u2 = nc.dram_tensor("w_u2", [2, P, D], BF16, kind="ExternalInput")
    b_m1 = nc.dram_tensor("b_m1", [P, 2], F32, kind="ExternalInput")
    b_m2 = nc.dram_tensor("b_m2", [P, 2], F32, kind="ExternalInput")
    b_u1 = nc.dram_tensor("b_u1", [P, 2], F32, kind="ExternalInput")
    b_u2r = nc.dram_tensor("b_u2r", [1, D], BF16, kind="ExternalInput")

    d1_src = nc.dram_tensor("d1_src", [P, C1 * 8], I16, kind="ExternalInput")
    d1_nw = nc.dram_tensor("d1_nw", [P, C1, H], F32, kind="ExternalInput")
    d1_oh = nc.dram_tensor("d1_oh", [C1, P, 2 * P], BF16, kind="ExternalInput")
    d2_src = nc.dram_tensor("d2_src", [P, C2 * 8], I16, kind="ExternalInput")
    d2_nw = nc.dram_tensor("d2_nw", [P, C2, H], F32, kind="ExternalInput")
    d2_oh = nc.dram_tensor("d2_oh", [C2, P, 2 * P], BF16, kind="ExternalInput")

    out_inv = nc.dram_tensor("out_inv", [ISHP, D], F32, kind="ExternalOutput")
    out_ast = nc.dram_tensor("out_ast", [ASHP, D], F32, kind="ExternalOutput")

    info1 = _chunk_info(meta1)
    info2 = _chunk_info(meta2)

    with tile.TileContext(nc) as tc:
        with ExitStack() as ctx:
            wpool = ctx.enter_context(tc.tile_pool(name="w", bufs=1))
            hp = ctx.enter_context(tc.tile_pool(name="hp", bufs=3))
            tp = ctx.enter_context(tc.tile_pool(name="tp", bufs=3))
            op = ctx.enter_context(tc.tile_pool(name="op", bufs=3))
            gp = ctx.enter_context(tc.tile_pool(name="gp", bufs=2))
            sp = ctx.enter_context(tc.tile_pool(name="sp", bufs=2))
            ohp = ctx.enter_context(tc.tile_pool(name="ohp", bufs=4))
            qbp = ctx.enter_context(tc.tile_pool(name="qbp", bufs=2))
            ps_mm = ctx.enter_context(tc.tile_pool(name="ps_mm", bufs=2, space="PSUM"))
            ps_nu = ctx.enter_context(tc.tile_pool(name="ps_nu", bufs=2, space="PSUM"))
            ps_qe = ctx.enter_context(tc.tile_pool(name="ps_qe", bufs=2, space="PSUM"))
            dram = ctx.enter_context(tc.tile_pool(name="dram", bufs=1, space="DRAM"))

            ones_t = wpool.tile([1, P], BF16, tag="ones_t")
            nc.vector.memset(ones_t[:], 1.0)
            z256 = wpool.tile([P, D], F32, tag="z256")
            nc.vector.memset(z256[:], 0.0)
            z4 = wpool.tile([P, H], F32, tag="z4")
            nc.vector.memset(z4[:], 0.0)
            z256b = wpool.tile([P, D], BF16, tag="z256b")
            nc.vector.memset(z256b[:], 0.0)

            def load_w(dram_w, nk, nd, dt, tag):
                t = wpool.tile([P, nk, nd], dt, tag=tag)
                for k in range(nk):
                    nc.sync.dma_start(t[:, k, :], dram_w[k, :, :])
                return t

            m1_t = load_w(w_m1, 2, D, BF16, "wm1")
            m2_t = load_w(w_m2, 2, D, BF16, "wm2")
            qk_t = load_w(w_qk, 2, D2, BF16, "wqk")
            v_t = load_w(w_v, 2, D, BF16, "wv")
            u1_t = load_w(w_u1, 4, D, BF16, "wu1")
            u2_t = load_w(w_u2, 2, D, BF16, "wu2")

            def load_b(dram_b, tag):
                t = wpool.tile([P, 2], F32, tag=tag)
                nc.sync.dma_start(t[:], dram_b[:, :])
                return t

            bm1_t, bm2_t, bu1_t = load_b(b_m1, "bm1"), load_b(b_m2, "bm2"), load_b(b_u1, "bu1")
            bu2_t = wpool.tile([1, D], BF16, tag="bu2")
            nc.sync.dma_start(bu2_t[:], b_u2r[:, :])

            q_inv = dram.tile([ISHP, D], BF16, tag="q_inv")
            kv_inv = dram.tile([ISHP, D2], BF16, tag="kv_inv")
            q_a_sh = dram.tile([ASHP, D], BF16, tag="q_a_sh")
            kv_a_sh = dram.tile([ASHP, D2], BF16, tag="kv_a_sh")
            q_a_full = dram.tile([ASHP * NC, D], BF16, tag="q_a_full")
            kv_a_full = dram.tile([ASHP * NC, D2], BF16, tag="kv_a_full")
            numer_d = dram.tile([ASHP * NC, D], F32, tag="numer_d")
            sumex_d = dram.tile([ASHP * NC, H], F32, tag="sumex_d")
            numer_sh = dram.tile([ASHP, D], F32, tag="numer_sh")
            sumex_sh = dram.tile([ASHP, H], F32, tag="sumex_sh")
            msg_inv = dram.tile([ISHP, D], BF16, tag="msg_inv")
            msg_ast = dram.tile([ASHP, D], BF16, tag="msg_ast")

            # ================= Phase A =================
            def phase_a(hT_in, ntiles, q_dram, kv_dram):
                for t in range(ntiles):
                    hTb = tp.tile([P, 2, P], BF16, tag="hTb")
                    nc.sync.dma_start(hTb[:], hT_in[t * P:(t + 1) * P, :])

                    def ff_layer(inT, w_tile, b_tile, tag):
                        outT = tp.tile([P, 2, P], BF16, tag=tag)
                        for hf in range(2):
                            pm = ps_mm.tile([P, D2], F32, tag="pm")
                            for k in range(2):
                                nc.tensor.matmul(
                                    pm[:, 0:P],
                                    lhsT=w_tile[:, k, hf * P:(hf + 1) * P],
                                    rhs=inT[:, k, :],
                                    start=(k == 0), stop=(k == 1))
                            nc.scalar.activation(
                                outT[:, hf, :], pm[:, 0:P], AF.Gelu,
                                bias=b_tile[:, hf:hf + 1])
                        return outT

                    mT1 = ff_layer(hTb, m1_t, bm1_t, "mT1")
                    mT2 = ff_layer(mT1, m2_t, bm2_t, "mT2")

                    pqk = ps_mm.tile([P, D2], F32, tag="pm")
                    for k in range(2):
                        nc.tensor.matmul(pqk[:], lhsT=hTb[:, k, :],
                                         rhs=qk_t[:, k, :],
                                         start=(k == 0), stop=(k == 1))
                    oq = op.tile([P, D], BF16, tag="proj_q")
                    nc.vector.tensor_copy(oq[:], pqk[:, 0:D])
                    nc.scalar.dma_start(q_dram[t * P:(t + 1) * P, :], oq[:])
                    ok_ = op.tile([P, D], BF16, tag="proj_k")
                    nc.vector.tensor_copy(ok_[:], pqk[:, D:D2])
                    nc.scalar.dma_start(kv_dram[t * P:(t + 1) * P, 0:D], ok_[:])
                    pv = ps_mm.tile([P, D2], F32, tag="pm")
                    for k in range(2):
                        nc.tensor.matmul(pv[:, 0:D], lhsT=mT2[:, k, :],
                                         rhs=v_t[:, k, :],
                                         start=(k == 0), stop=(k == 1))
                    ov = op.tile([P, D], BF16, tag="proj_v")
                    nc.vector.tensor_copy(ov[:], pv[:, 0:D])
                    nc.scalar.dma_start(kv_dram[t * P:(t + 1) * P, D:D2], ov[:])

            phase_a(hT_a_in, NT_A, q_a_sh, kv_a_sh)

            rg = [list(range(NC))]
            nc.gpsimd.collective_compute(
                "AllGather", ALU.bypass, replica_groups=rg,
                ins=[kv_a_sh.opt()], outs=[kv_a_full.opt()])
            nc.gpsimd.collective_compute(
                "AllGather", ALU.bypass, replica_groups=rg,
                ins=[q_a_sh.opt()], outs=[q_a_full.opt()])

            phase_a(hT_i_in, NT_I, q_inv, kv_inv)

            # ================= Phase B =================
            last_exp = [None]

            def phase_b(info, C, meta, src_sb, nw_sb, oh_dram,
                        q_tbl, kv_tbl, sink):
                numer = [None]
                qblk = None

                def flush(batch):
                    g0, n, ohg, exe = batch
                    for j in range(n):
                        c = g0 + j
                        blk, first, last = info[c]
                        if first:
                            numer[0] = ps_nu.tile([P, D + H], F32, tag="nu", name="nu")
                        nu_ps = numer[0]
                        nc.tensor.matmul(nu_ps[:], lhsT=ohg[:, j, P:2 * P],
                                         rhs=exe[:, j, :],
                                         start=first, stop=last)
                        if last:
                            rows = slice(blk * P, (blk + 1) * P)
                            if sink[0] == "msg":
                                den = sp.tile([P, H], F32, tag="den")
                                nc.vector.tensor_scalar(
                                    den[:], nu_ps[:, D:D + H], 1e-10, None,
                                    ALU.add)
                                rec = sp.tile([P, H], F32, tag="rec")
                                nc.vector.reciprocal(rec[:], den[:])
                                msg = op.tile([P, D], BF16, tag="msg")
                                nc.vector.tensor_tensor(
                                    msg[:].rearrange("p (h k) -> p h k", h=H),
                                    nu_ps[:, 0:D].rearrange("p (h k) -> p h k", h=H),
                                    rec[:].unsqueeze(-1).to_broadcast((P, H, DK)),
                                    ALU.mult)
                                nc.scalar.dma_start(sink[1][rows, :], msg[:])
                            else:
                                nu_sb = op.tile([P, D], F32, tag="nu_sb")
                                nc.vector.tensor_copy(nu_sb[:], nu_ps[:, 0:D])
                                nc.scalar.dma_start(sink[1][rows, :], nu_sb[:])
                                se_sb = sp.tile([P, H], F32, tag="se_sb")
                                nc.vector.tensor_copy(se_sb[:], nu_ps[:, D:D + H])
                                nc.scalar.dma_start(sink[2][rows, :], se_sb[:])

                pend = None
                for g0 in range(0, C, SUBB):
                    n = min(SUBB, C - g0)
                    kvg = gp.tile([P, SUBB, D2], BF16, tag="kvg")
                    nc.gpsimd.dma_gather(
                        out_ap=kvg[:, 0:n, :], in_ap=kv_tbl,
                        idxs_ap=src_sb[:, g0 * 8:(g0 + n) * 8],
                        num_idxs=n * P, num_idxs_reg=n * P, elem_size=D2)
                    ohg = ohp.tile([P, SUBB, 2 * P], BF16, tag="ohg")
                    nc.sync.dma_start(
                        ohg[:, 0:n, :],
                        oh_dram[g0:g0 + n].transpose([1, 0, 2]))
                    qk = sp.tile([P, SUBB, D], F32, tag="qk")
                    for j in range(n):
                        c = g0 + j
                        blk, first, last = info[c]
                        if first:
                            qblk = qbp.tile([P, D], BF16, tag="qblk")
                            nc.sync.dma_start(
                                qblk[:], q_tbl[blk * P:(blk + 1) * P, :])
                        qe = ps_qe.tile([P, D], F32, tag="qe")
                        nc.tensor.matmul(qe[:], lhsT=ohg[:, j, 0:P], rhs=qblk[:],
                                         start=True, stop=True)
                        nc.vector.tensor_tensor(
                            qk[:, j, :], qe[:], kvg[:, j, 0:D], ALU.mult)
                    s4 = sp.tile([P, SUBB, H], F32, tag="s4")
                    nc.vector.tensor_reduce(
                        s4[:, 0:n, :],
                        qk[:, 0:n, :].rearrange("p c (h k) -> p c h k", h=H),
                        axis=mybir.AxisListType.X, op=ALU.add)
                    ex0 = sp.tile([P, SUBB, H], F32, tag="ex0")
                    last_exp[0] = nc.scalar.activation(
                        ex0[:, 0:n, :], s4[:, 0:n, :], AF.Exp, scale=0.125)
                    exb = sp.tile([P, SUBB, H], F32, tag="exb")
                    nc.vector.tensor_tensor(
                        exb[:, 0:n, :], ex0[:, 0:n, :], nw_sb[:, g0:g0 + n, :],
                        ALU.mult)
                    exe = sp.tile([P, SUBB, D + H], BF16, tag="exe", bufs=3)
                    nc.vector.tensor_tensor(
                        exe[:, 0:n, 0:D].rearrange("p c (h k) -> p c h k", h=H),
                        kvg[:, 0:n, D:D2].rearrange("p c (h k) -> p c h k", h=H),
                        exb[:, 0:n, :].unsqueeze(-1).to_broadcast((P, n, H, DK)),
                        ALU.mult)
                    nc.vector.tensor_copy(exe[:, 0:n, D:D + H], exb[:, 0:n, :])
                    if pend is not None:
                        flush(pend)
                    pend = (g0, n, ohg, exe)
                if pend is not None:
                    flush(pend)
                for b, k in enumerate(meta):
                    if k != 0:
                        continue
                    rows = slice(b * P, (b + 1) * P)
                    if sink[0] == "msg":
                        nc.sync.dma_start(sink[1][rows, :], z256b[:])
                    else:
                        nc.sync.dma_start(sink[1][rows, :], z256[:])
                        nc.sync.dma_start(sink[2][rows, :], z4[:])

            d1_src_sb = wpool.tile([P, C1 * 8], I16, tag="d1_src_sb")
            nc.sync.dma_start(d1_src_sb[:], d1_src[:, :])
            d1_nw_sb = wpool.tile([P, C1, H], F32, tag="d1_nw_sb")
            nc.sync.dma_start(d1_nw_sb[:], d1_nw[:, :, :])
            d2_src_sb = wpool.tile([P, C2 * 8], I16, tag="d2_src_sb")
            nc.sync.dma_start(d2_src_sb[:], d2_src[:, :])
            d2_nw_sb = wpool.tile([P, C2, H], F32, tag="d2_nw_sb")
            nc.sync.dma_start(d2_nw_sb[:], d2_nw[:, :, :])

            phase_b(info1, C1, meta1, d1_src_sb, d1_nw_sb, d1_oh,
                    q_inv[:, :], kv_a_full[:, :], ("msg", msg_inv))
            phase_b(info2, C2, meta2, d2_src_sb, d2_nw_sb, d2_oh,
                    q_a_full[:, :], kv_inv[:, :], ("acc", numer_d, sumex_d))

            nc.gpsimd.collective_compute(
                "ReduceScatter", ALU.add, replica_groups=rg,
                ins=[numer_d.opt()], outs=[numer_sh.opt()])
            nc.gpsimd.collective_compute(
                "ReduceScatter", ALU.add, replica_groups=rg,
                ins=[sumex_d.opt()], outs=[sumex_sh.opt()])

            # ================= Phase C =================
            first_gelu = [None]

            def phase_c(ntiles, hT_in, msg_dram, out_dram):
                for t in range(ntiles):
                    cat = tp.tile([P, 4, P], BF16, tag="cat")
                    nc.sync.dma_start(cat[:, 0:2, :], hT_in[t * P:(t + 1) * P, :])
                    for k in range(2):
                        nc.sync.dma_start_transpose(
                            cat[:, 2 + k, :],
                            msg_dram[t * P:(t + 1) * P, k * P:(k + 1) * P])
                    y1 = tp.tile([P, 2, P], BF16, tag="y1")
                    pm = ps_mm.tile([P, D], F32, tag="pmc")
                    for hf in range(2):
                        for k in range(4):
                            nc.tensor.matmul(
                                pm[:, hf * P:(hf + 1) * P],
                                lhsT=u1_t[:, k, hf * P:(hf + 1) * P],
                                rhs=cat[:, k, :],
                                start=(k == 0), stop=(k == 3))
                    for hf in range(2):
                        g = nc.scalar.activation(
                            y1[:, hf, :], pm[:, hf * P:(hf + 1) * P], AF.Gelu,
                            bias=bu1_t[:, hf:hf + 1])
                        if first_gelu[0] is None:
                            first_gelu[0] = g
                            if last_exp[0] is not None:
                                add_dep_helper(
                                    g.ins, last_exp[0].ins,
                                    reason="gelu after exp (ACT tables)")
                    po = ps_mm.tile([P, D], F32, tag="pmc")
                    for k in range(2):
                        nc.tensor.matmul(po[:], lhsT=y1[:, k, :],
                                         rhs=u2_t[:, k, :],
                                         start=(k == 0), stop=False)
                    nc.tensor.matmul(po[:], lhsT=ones_t[0:1, :],
                                     rhs=bu2_t[0:1, :], start=False, stop=True)
                    ot = op.tile([P, D], F32, tag="fin")
                    nc.scalar.activation(ot[:], po[:], AF.Gelu)
                    nc.scalar.dma_start(out_dram[t * P:(t + 1) * P, :], ot[:])

            phase_c(NT_I, hT_i_in, msg_inv, out_inv)

            for t in range(NT_A):
                nu = hp.tile([P, D], F32, tag="nu_f")
                nc.sync.dma_start(nu[:], numer_sh[t * P:(t + 1) * P, :])
                se = sp.tile([P, H], F32, tag="se_f")
                nc.sync.dma_start(se[:], sumex_sh[t * P:(t + 1) * P, :])
                den = sp.tile([P, H], F32, tag="den")
                nc.vector.tensor_scalar(den[:], se[:], 1e-10, None, ALU.add)
                rec = sp.tile([P, H], F32, tag="rec")
                nc.vector.reciprocal(rec[:], den[:])
                msg = op.tile([P, D], BF16, tag="msg")
                nc.vector.tensor_tensor(
                    msg[:].rearrange("p (h k) -> p h k", h=H),
                    nu[:].rearrange("p (h k) -> p h k", h=H),
                    rec[:].unsqueeze(-1).to_broadcast((P, H, DK)), ALU.mult)
                nc.sync.dma_start(msg_ast[t * P:(t + 1) * P, :], msg[:])

            phase_c(NT_A, hT_a_in, msg_ast, out_ast)

    nc.compile()
    return nc


# ----------------------------------------------------------------------------
# Entry point
# ----------------------------------------------------------------------------

def kernel(inv_h, asset_h, inv_norm_w, asset_norm_w,
           m_w1, m_b1, m_w2, m_b2, Wq, Wk, Wv,
           u_w1, u_b1, u_w2, u_b2, edge_tgt, edge_src):
    global _LAST_EXEC_NS
    bf = ml_dtypes.bfloat16
    inv_h = np.asarray(inv_h, np.float32)
    asset_h = np.asarray(asset_h, np.float32)
    inv_norm_w = np.asarray(inv_norm_w, np.float32)
    asset_norm_w = np.asarray(asset_norm_w, np.float32)
    edge_tgt = np.asarray(edge_tgt).astype(np.int64)
    edge_src = np.asarray(edge_src).astype(np.int64)
    m_w1, m_b1 = np.asarray(m_w1, np.float32), np.asarray(m_b1, np.float32)
    m_w2, m_b2 = np.asarray(m_w2, np.float32), np.asarray(m_b2, np.float32)
    Wq, Wk, Wv = (np.asarray(x, np.float32) for x in (Wq, Wk, Wv))
    u_w1, u_b1 = np.asarray(u_w1, np.float32), np.asarray(u_b1, np.float32)
    u_w2, u_b2 = np.asarray(u_w2, np.float32), np.asarray(u_b2, np.float32)

    ast_row = (edge_src // ASH) * ASHP + (edge_src % ASH)
    inv_core = edge_tgt // ISH
    inv_loc = edge_tgt - inv_core * ISH

    meta1, C1, d1 = _bucket(
        core=inv_core, blk=inv_loc // P, n_blocks=NT_I,
        srcidx=ast_row, t128_all=inv_loc % P, nw=inv_norm_w)
    meta2, C2, d2 = _bucket(
        core=inv_core, blk=ast_row // P, n_blocks=NB2,
        srcidx=inv_loc, t128_all=ast_row % P, nw=asset_norm_w)

    nc = _build(meta1, C1, meta2, C2)

    w_qk_h = np.concatenate([Wq.reshape(2, P, D), Wk.reshape(2, P, D)], axis=2)
    common = {
        "w_m1": m_w1.reshape(2, P, D).astype(bf),
        "w_m2": m_w2.reshape(2, P, D).astype(bf),
        "w_qk": w_qk_h.astype(bf),
        "w_v": Wv.reshape(2, P, D).astype(bf),
        "w_u1": u_w1.reshape(4, P, D).astype(bf),
        "w_u2": u_w2.reshape(2, P, D).astype(bf),
        "b_m1": m_b1.reshape(2, P).T.copy(),
        "b_m2": m_b2.reshape(2, P).T.copy(),
        "b_u1": u_b1.reshape(2, P).T.copy(),
        "b_u2r": u_b2.reshape(1, D).astype(bf),
    }

    in_maps = []
    for c in range(NC):
        s1, t1, n1 = d1[c]
        s2, t2, n2 = d2[c]
        m = dict(common)
        m["hT_i_in"] = _hT(_pad_rows(inv_h[c * ISH:(c + 1) * ISH], ISHP), NT_I)
        m["hT_a_in"] = _hT(_pad_rows(asset_h[c * ASH:(c + 1) * ASH], ASHP), NT_A)
        m["d1_src"] = _wrap16(s1, C1)
        m["d1_nw"] = _colmajor(n1, C1, rep=H)
        m["d1_oh"] = _onehots(t1, C1)
        m["d2_src"] = _wrap16(s2, C2)
        m["d2_nw"] = _colmajor(n2, C2, rep=H)
        m["d2_oh"] = _onehots(t2, C2)
        in_maps.append(m)

    res = bass_utils.run_bass_kernel_spmd(
        nc, in_maps, core_ids=list(range(NC)), trace=True)
    _LAST_EXEC_NS = res.exec_time_ns

    inv_out = np.concatenate(
        [res.results[c]["out_inv"][:ISH] for c in range(NC)], axis=0)
    ast_out = np.concatenate(
        [res.results[c]["out_ast"][:ASH] for c in range(NC)], axis=0)
    return inv_out, ast_out


# revision 14
# speedup vs baseline: 1.5643x; 1.0081x over previous
"""Trainium2 Bass kernel for nn_MessagePassingLayer (bipartite GNN attention
message passing), distributed over 8 NeuronCores.

v5 design notes:
  - Node tables row-sharded 8 ways (inv 6250->6272 padded, asset 1250->1280).
  - dma_gather on TRN2 is descriptor-generation bound (~5.9ns/idx + 2.6us/call,
    independent of row bytes), so K|V are packed into one fp32 [N,512] row
    (one gather per edge total) and Q is never gathered: per target block the
    128 Q rows are densely loaded and Qe is formed on the PE with a
    host-precomputed transposed one-hot (exact in bf16).
  - Phase A: project local shard; FF runs feature-major off a host-supplied
    transposed bf16 copy of h, so there are no on-device transposes.  Small
    asset tables (kv_a fp32, q_a bf16) are AllGathered; the 51MB investor
    tables stay local (dir2 is shard-by-source + ReduceScatter of the segment
    stats instead).
  - Phase B per 128-edge chunk: gather KV[src]; Qe = oh_te @ Qblk (PE);
    qk = Qe*Ke (DVE, fp32); per-head reduce; ex = nw*exp(s/8) (max-term
    dropped -- exact to 1e-10 rel here: ex equals the reference's exactly
    when the segment max < 0, and the 1e-10 eps is negligible otherwise);
    exe=[ex*V | ex] in bf16; numer|sumex accumulated per target block by one
    one-hot matmul into fp32 PSUM; divide at block end.  Batched DVE ops
    amortize op overhead across 8-chunk gather batches.
  - dir2 partial numer/sumex (over all 10240 asset rows) are ReduceScattered.
  - Phase C: out = FF([h, msg]); msg transposed via hardware DMA-transpose
    (bf16); matmuls bf16, final gelu fp32 out.
  - gelu and exp live in different ACT table sets; an explicit dep keeps all
    phase-C gelus after the last phase-B exp to avoid table-reload thrash.
"""

import ml_dtypes
import numpy as np
from contextlib import ExitStack

import concourse.bass as bass
import concourse.tile as tile
from concourse.tile import add_dep_helper
from concourse import bacc, mybir
from concourse import bass_utils

F32 = mybir.dt.float32
BF16 = mybir.dt.bfloat16
I16 = mybir.dt.int16
AF = mybir.ActivationFunctionType
ALU = mybir.AluOpType

I_N, A_N, E_N = 50000, 10000, 200000
D, H, DK = 256, 4, 64
D2 = 2 * D
NC = 8
P = 128
ISH, ASH = I_N // NC, A_N // NC            # 6250, 1250
ISHP, ASHP = 6272, 1280
NT_I, NT_A = ISHP // P, ASHP // P          # 49, 10
NB2 = ASHP * NC // P                       # 80 global asset blocks
SUBB = 8                                   # chunks per gather call (1024 idx cap)

_LAST_EXEC_NS = None


# ----------------------------------------------------------------------------
# Host-side edge preparation
# ----------------------------------------------------------------------------

def _bucket(core, blk, n_blocks, srcidx, t128_all, nw):
    buckets = {}
    for c in range(NC):
        m_c = core == c
        for b in range(n_blocks):
            buckets[(c, b)] = np.nonzero(m_c & (blk == b))[0]
    meta = [max((len(buckets[(c, b)]) + P - 1) // P for c in range(NC))
            for b in range(n_blocks)]
    C = sum(meta)
    cores_out = []
    for c in range(NC):
        s16 = np.zeros(C * P, np.int64)
        t128 = np.full(C * P, -1, np.int64)
        nwv = np.zeros(C * P, np.float32)
        pos = 0
        for b in range(n_blocks):
            idx = buckets[(c, b)]
            n = len(idx)
            if meta[b] == 0:
                continue
            sl = slice(pos, pos + n)
            s16[sl] = srcidx[idx]
            t128[sl] = t128_all[idx]
            nwv[sl] = nw[idx]
            pos += meta[b] * P
        assert pos == C * P
        cores_out.append((s16, t128, nwv))
    return meta, C, cores_out


def _wrap16(flat_idx, C):
    assert flat_idx.max(initial=0) < 32768 and flat_idx.min(initial=0) >= 0
    w = flat_idx.astype(np.int16).reshape(C * 8, 16).T
    return np.tile(w, (8, 1)).copy()


def _colmajor(flat, C, rep=1):
    a = flat.reshape(C, P).T.copy()
    if rep == 1:
        return a
    return np.repeat(a[:, :, None], rep, axis=2).copy()


def _onehots(t128, C):
    """[C*128] targets (-1=pad) -> combined [C, 128, 256] = [oh_te | oh_et]."""
    bf = ml_dtypes.bfloat16
    t = t128.reshape(C, P, 1)
    oh_et = (t == np.arange(P).reshape(1, 1, P)).astype(bf)
    out = np.empty((C, P, 2 * P), bf)
    out[:, :, 0:P] = oh_et.transpose(0, 2, 1)
    out[:, :, P:2 * P] = oh_et
    return out


def _pad_rows(a, n):
    out = np.zeros((n, a.shape[1]), a.dtype)
    out[: a.shape[0]] = a
    return out


def _hT(h_pad, ntiles):
    """[N,256] f32 -> bf16 transposed-tile layout [N,256]:
    row (t*128+f), col (k*128+n) = h[t*128+n, k*128+f]."""
    bf = ml_dtypes.bfloat16
    x = h_pad.reshape(ntiles, P, 2, P)          # (t, n, k, f)
    x = x.transpose(0, 3, 2, 1)                 # (t, f, k, n)
    return np.ascontiguousarray(x.reshape(ntiles * P, D).astype(bf))


def _chunk_info(meta):
    info = []
    for b, k in enumerate(meta):
        for j in range(k):
            info.append((b, j == 0, j == k - 1))
    return info


# ----------------------------------------------------------------------------
# Device program
# ----------------------------------------------------------------------------

def _build(meta1, C1, meta2, C2):
    nc = bacc.Bacc("TRN2", target_bir_lowering=False, debug=False,
                   enable_asserts=True, num_devices=NC)

    hT_i_in = nc.dram_tensor("hT_i_in", [NT_I * P, D], BF16, kind="ExternalInput")
    hT_a_in = nc.dram_tensor("hT_a_in", [NT_A * P, D], BF16, kind="ExternalInput")
    w_m1 = nc.dram_tensor("w_m1", [2, P, D], BF16, kind="ExternalInput")
    w_m2 = nc.dram_tensor("w_m2", [2, P, D], BF16, kind="ExternalInput")
    w_qk = nc.dram_tensor("w_qk", [2, P, D2], BF16, kind="ExternalInput")
    w_v = nc.dram_tensor("w_v", [2, P, D], BF16, kind="ExternalInput")
    w_u1 = nc.dram_tensor("w_u1", [4, P, D], BF16, kind="ExternalInput")
    w_u2 = nc.dram_tensor("w_u2", [2, P, D], BF16, kind="ExternalInput")
    b_m1 = nc.dram_tensor("b_m1", [P, 2], F32, kind="ExternalInput")
    b_m2 = nc.dram_tensor("b_m2", [P, 2], F32, kind="ExternalInput")
    b_u1 = nc.dram_tensor("b_u1", [P, 2], F32, kind="ExternalInput")
    b_u2r = nc.dram_tensor("b_u2r", [1, D], BF16, kind="ExternalInput")

    d1_src = nc.dram_tensor("d1_src", [P, C1 * 8], I16, kind="ExternalInput")
    d1_nw = nc.dram_tensor("d1_nw", [P, C1, H], F32, kind="ExternalInput")
    d1_oh = nc.dram_tensor("d1_oh", [C1, P, 2 * P], BF16, kind="ExternalInput")
    d2_src = nc.dram_tensor("d2_src", [P, C2 * 8], I16, kind="ExternalInput")
    d2_nw = nc.dram_tensor("d2_nw", [P, C2, H], F32, kind="ExternalInput")
    d2_oh = nc.dram_tensor("d2_oh", [C2, P, 2 * P], BF16, kind="ExternalInput")

    out_inv = nc.dram_tensor("out_inv", [ISHP, D], F32, kind="ExternalOutput")
    out_ast = nc.dram_tensor("out_ast", [ASHP, D], F32, kind="ExternalOutput")

    info1 = _chunk_info(meta1)
    info2 = _chunk_info(meta2)

    with tile.TileContext(nc) as tc:
        with ExitStack() as ctx:
            wpool = ctx.enter_context(tc.tile_pool(name="w", bufs=1))
            hp = ctx.enter_context(tc.tile_pool(name="hp", bufs=3))
            tp = ctx.enter_context(tc.tile_pool(name="tp", bufs=3))
            op = ctx.enter_context(tc.tile_pool(name="op", bufs=3))
            gp = ctx.enter_context(tc.tile_pool(name="gp", bufs=2))
            sp = ctx.enter_context(tc.tile_pool(name="sp", bufs=2))
            ohp = ctx.enter_context(tc.tile_pool(name="ohp", bufs=4))
            qbp = ctx.enter_context(tc.tile_pool(name="qbp", bufs=2))
            ps_mm = ctx.enter_context(tc.tile_pool(name="ps_mm", bufs=2, space="PSUM"))
            ps_nu = ctx.enter_context(tc.tile_pool(name="ps_nu", bufs=2, space="PSUM"))
            ps_qe = ctx.enter_context(tc.tile_pool(name="ps_qe", bufs=2, space="PSUM"))
            dram = ctx.enter_context(tc.tile_pool(name="dram", bufs=1, space="DRAM"))

            ones_t = wpool.tile([1, P], BF16, tag="ones_t")
            nc.vector.memset(ones_t[:], 1.0)
            z256 = wpool.tile([P, D], F32, tag="z256")
            nc.vector.memset(z256[:], 0.0)
            z4 = wpool.tile([P, H], F32, tag="z4")
            nc.vector.memset(z4[:], 0.0)
            z256b = wpool.tile([P, D], BF16, tag="z256b")
            nc.vector.memset(z256b[:], 0.0)

            def load_w(dram_w, nk, nd, dt, tag):
                t = wpool.tile([P, nk, nd], dt, tag=tag)
                for k in range(nk):
                    nc.sync.dma_start(t[:, k, :], dram_w[k, :, :])
                return t

            m1_t = load_w(w_m1, 2, D, BF16, "wm1")
            m2_t = load_w(w_m2, 2, D, BF16, "wm2")
            qk_t = load_w(w_qk, 2, D2, BF16, "wqk")
            v_t = load_w(w_v, 2, D, BF16, "wv")
            u1_t = load_w(w_u1, 4, D, BF16, "wu1")
            u2_t = load_w(w_u2, 2, D, BF16, "wu2")

            def load_b(dram_b, tag):
                t = wpool.tile([P, 2], F32, tag=tag)
                nc.sync.dma_start(t[:], dram_b[:, :])
                return t

            bm1_t, bm2_t, bu1_t = load_b(b_m1, "bm1"), load_b(b_m2, "bm2"), load_b(b_u1, "bu1")
            bu2_t = wpool.tile([1, D], BF16, tag="bu2")
            nc.sync.dma_start(bu2_t[:], b_u2r[:, :])

            q_inv = dram.tile([ISHP, D], BF16, tag="q_inv")
            kv_inv = dram.tile([ISHP, D2], BF16, tag="kv_inv")
            q_a_sh = dram.tile([ASHP, D], BF16, tag="q_a_sh")
            kv_a_sh = dram.tile([ASHP, D2], BF16, tag="kv_a_sh")
            q_a_full = dram.tile([ASHP * NC, D], BF16, tag="q_a_full")
            kv_a_full = dram.tile([ASHP * NC, D2], BF16, tag="kv_a_full")
            numer_d = dram.tile([ASHP * NC, D], F32, tag="numer_d")
            sumex_d = dram.tile([ASHP * NC, H], F32, tag="sumex_d")
            numer_sh = dram.tile([ASHP, D], F32, tag="numer_sh")
            sumex_sh = dram.tile([ASHP, H], F32, tag="sumex_sh")
            msg_inv = dram.tile([ISHP, D], BF16, tag="msg_inv")
            msg_ast = dram.tile([ASHP, D], BF16, tag="msg_ast")

            # ================= Phase A =================
            def phase_a(hT_in, ntiles, q_dram, kv_dram):
                for t in range(ntiles):
                    hTb = tp.tile([P, 2, P], BF16, tag="hTb")
                    nc.sync.dma_start(hTb[:], hT_in[t * P:(t + 1) * P, :])

                    def ff_layer(inT, w_tile, b_tile, tag):
                        outT = tp.tile([P, 2, P], BF16, tag=tag)
                        for hf in range(2):
                            pm = ps_mm.tile([P, D2], F32, tag="pm")
                            for k in range(2):
                                nc.tensor.matmul(
                                    pm[:, 0:P],
                                    lhsT=w_tile[:, k, hf * P:(hf + 1) * P],
                                    rhs=inT[:, k, :],
                                    start=(k == 0), stop=(k == 1))
                            nc.scalar.activation(
                                outT[:, hf, :], pm[:, 0:P], AF.Gelu,
                                bias=b_tile[:, hf:hf + 1])
                        return outT

                    mT1 = ff_layer(hTb, m1_t, bm1_t, "mT1")
                    mT2 = ff_layer(mT1, m2_t, bm2_t, "mT2")

                    pqk = ps_mm.tile([P, D2], F32, tag="pm")
                    for k in range(2):
                        nc.tensor.matmul(pqk[:], lhsT=hTb[:, k, :],
                                         rhs=qk_t[:, k, :],
                                         start=(k == 0), stop=(k == 1))
                    oq = op.tile([P, D], BF16, tag="proj_q")
                    nc.vector.tensor_copy(oq[:], pqk[:, 0:D])
                    nc.sync.dma_start(q_dram[t * P:(t + 1) * P, :], oq[:])
                    ok_ = op.tile([P, D], BF16, tag="proj_k")
                    nc.vector.tensor_copy(ok_[:], pqk[:, D:D2])
                    nc.sync.dma_start(kv_dram[t * P:(t + 1) * P, 0:D], ok_[:])
                    pv = ps_mm.tile([P, D2], F32, tag="pm")
                    for k in range(2):
                        nc.tensor.matmul(pv[:, 0:D], lhsT=mT2[:, k, :],
                                         rhs=v_t[:, k, :],
                                         start=(k == 0), stop=(k == 1))
                    ov = op.tile([P, D], BF16, tag="proj_v")
                    nc.vector.tensor_copy(ov[:], pv[:, 0:D])
                    nc.scalar.dma_start(kv_dram[t * P:(t + 1) * P, D:D2], ov[:])

            phase_a(hT_a_in, NT_A, q_a_sh, kv_a_sh)

            rg = [list(range(NC))]
            nc.gpsimd.collective_compute(
                "AllGather", ALU.bypass, replica_groups=rg,
                ins=[kv_a_sh.opt()], outs=[kv_a_full.opt()])
            nc.gpsimd.collective_compute(
                "AllGather", ALU.bypass, replica_groups=rg,
                ins=[q_a_sh.opt()], outs=[q_a_full.opt()])

            phase_a(hT_i_in, NT_I, q_inv, kv_inv)

            # ================= Phase B =================
            last_exp = [None]

            def phase_b(info, C, meta, src_sb, nw_sb, oh_dram,
                        q_tbl, kv_tbl, sink):
                numer = [None]
                qblk = None

                def flush(batch):
                    g0, n, ohg, exe = batch
                    for j in range(n):
                        c = g0 + j
                        blk, first, last = info[c]
                        if first:
                            numer[0] = ps_nu.tile([P, D + H], F32, tag="nu", name="nu")
                        nu_ps = numer[0]
                        nc.tensor.matmul(nu_ps[:], lhsT=ohg[:, j, P:2 * P],
                                         rhs=exe[:, j, :],
                                         start=first, stop=last)
                        if last:
                            rows = slice(blk * P, (blk + 1) * P)
                            if sink[0] == "msg":
                                den = sp.tile([P, H], F32, tag="den")
                                nc.vector.tensor_scalar(
                                    den[:], nu_ps[:, D:D + H], 1e-10, None,
                                    ALU.add)
                                rec = sp.tile([P, H], F32, tag="rec")
                                nc.vector.reciprocal(rec[:], den[:])
                                msg = op.tile([P, D], BF16, tag="msg")
                                nc.vector.tensor_tensor(
                                    msg[:].rearrange("p (h k) -> p h k", h=H),
                                    nu_ps[:, 0:D].rearrange("p (h k) -> p h k", h=H),
                                    rec[:].unsqueeze(-1).to_broadcast((P, H, DK)),
                                    ALU.mult)
                                nc.sync.dma_start(sink[1][rows, :], msg[:])
                            else:
                                nu_sb = op.tile([P, D], F32, tag="nu_sb")
                                nc.vector.tensor_copy(nu_sb[:], nu_ps[:, 0:D])
                                nc.sync.dma_start(sink[1][rows, :], nu_sb[:])
                                se_sb = sp.tile([P, H], F32, tag="se_sb")
                                nc.vector.tensor_copy(se_sb[:], nu_ps[:, D:D + H])
                                nc.sync.dma_start(sink[2][rows, :], se_sb[:])

                pend = None
                for g0 in range(0, C, SUBB):
                    n = min(SUBB, C - g0)
                    kvg = gp.tile([P, SUBB, D2], BF16, tag="kvg", bufs=4)
                    nc.gpsimd.dma_gather(
                        out_ap=kvg[:, 0:n, :], in_ap=kv_tbl,
                        idxs_ap=src_sb[:, g0 * 8:(g0 + n) * 8],
                        num_idxs=n * P, num_idxs_reg=n * P, elem_size=D2)
                    ohg = ohp.tile([P, SUBB, 2 * P], BF16, tag="ohg")
                    nc.sync.dma_start(
                        ohg[:, 0:n, :],
                        oh_dram[g0:g0 + n].transpose([1, 0, 2]))
                    qk = sp.tile([P, SUBB, D], F32, tag="qk", bufs=3)
                    for j in range(n):
                        c = g0 + j
                        blk, first, last = info[c]
                        if first:
                            qblk = qbp.tile([P, D], BF16, tag="qblk")
                            nc.sync.dma_start(
                                qblk[:], q_tbl[blk * P:(blk + 1) * P, :])
                        qe = ps_qe.tile([P, D], F32, tag="qe")
                        nc.tensor.matmul(qe[:], lhsT=ohg[:, j, 0:P], rhs=qblk[:],
                                         start=True, stop=True)
                        nc.vector.tensor_tensor(
                            qk[:, j, :], qe[:], kvg[:, j, 0:D], ALU.mult)
                    s4 = sp.tile([P, SUBB, H], F32, tag="s4")
                    nc.vector.tensor_reduce(
                        s4[:, 0:n, :],
                        qk[:, 0:n, :].rearrange("p c (h k) -> p c h k", h=H),
                        axis=mybir.AxisListType.X, op=ALU.add)
                    ex0 = sp.tile([P, SUBB, H], F32, tag="ex0")
                    last_exp[0] = nc.scalar.activation(
                        ex0[:, 0:n, :], s4[:, 0:n, :], AF.Exp, scale=0.125)
                    exb = sp.tile([P, SUBB, H], F32, tag="exb")
                    nc.vector.tensor_tensor(
                        exb[:, 0:n, :], ex0[:, 0:n, :], nw_sb[:, g0:g0 + n, :],
                        ALU.mult)
                    exe = sp.tile([P, SUBB, D + H], BF16, tag="exe", bufs=3)
                    nc.vector.tensor_tensor(
                        exe[:, 0:n, 0:D].rearrange("p c (h k) -> p c h k", h=H),
                        kvg[:, 0:n, D:D2].rearrange("p c (h k) -> p c h k", h=H),
                        exb[:, 0:n, :].unsqueeze(-1).to_broadcast((P, n, H, DK)),
                        ALU.mult)
                    nc.vector.tensor_copy(exe[:, 0:n, D:D + H], exb[:, 0:n, :])
                    if pend is not None:
                        flush(pend)
                    pend = (g0, n, ohg, exe)
                if pend is not None:
                    flush(pend)
                for b, k in enumerate(meta):
                    if k != 0:
                        continue
                    rows = slice(b * P, (b + 1) * P)
                    if sink[0] == "msg":
                        nc.sync.dma_start(sink[1][rows, :], z256b[:])
                    else:
                        nc.sync.dma_start(sink[1][rows, :], z256[:])
                        nc.sync.dma_start(sink[2][rows, :], z4[:])

            d1_src_sb = wpool.tile([P, C1 * 8], I16, tag="d1_src_sb")
            nc.sync.dma_start(d1_src_sb[:], d1_src[:, :])
            d1_nw_sb = wpool.tile([P, C1, H], F32, tag="d1_nw_sb")
            nc.sync.dma_start(d1_nw_sb[:], d1_nw[:, :, :])
            d2_src_sb = wpool.tile([P, C2 * 8], I16, tag="d2_src_sb")
            nc.sync.dma_start(d2_src_sb[:], d2_src[:, :])
            d2_nw_sb = wpool.tile([P, C2, H], F32, tag="d2_nw_sb")
            nc.sync.dma_start(d2_nw_sb[:], d2_nw[:, :, :])

            phase_b(info1, C1, meta1, d1_src_sb, d1_nw_sb, d1_oh,
                    q_inv[:, :], kv_a_full[:, :], ("msg", msg_inv))
            phase_b(info2, C2, meta2, d2_src_sb, d2_nw_sb, d2_oh,
                    q_a_full[:, :], kv_inv[:, :], ("acc", numer_d, sumex_d))

            nc.gpsimd.collective_compute(
                "ReduceScatter", ALU.add, replica_groups=rg,
                ins=[numer_d.opt()], outs=[numer_sh.opt()])
            nc.gpsimd.collective_compute(
                "ReduceScatter", ALU.add, replica_groups=rg,
                ins=[sumex_d.opt()], outs=[sumex_sh.opt()])

            # ================= Phase C =================
            first_gelu = [None]

            def phase_c(ntiles, hT_in, msg_dram, out_dram):
                for t in range(ntiles):
                    cat = tp.tile([P, 4, P], BF16, tag="cat")
                    nc.sync.dma_start(cat[:, 0:2, :], hT_in[t * P:(t + 1) * P, :])
                    nc.sync.dma_start_transpose(
                        cat[:, 2, :], msg_dram[t * P:(t + 1) * P, 0:P])
                    nc.scalar.dma_start_transpose(
                        cat[:, 3, :], msg_dram[t * P:(t + 1) * P, P:D])
                    y1 = tp.tile([P, 2, P], BF16, tag="y1")
                    pm = ps_mm.tile([P, D], F32, tag="pmc")
                    for hf in range(2):
                        for k in range(4):
                            nc.tensor.matmul(
                                pm[:, hf * P:(hf + 1) * P],
                                lhsT=u1_t[:, k, hf * P:(hf + 1) * P],
                                rhs=cat[:, k, :],
                                start=(k == 0), stop=(k == 3))
                    for hf in range(2):
                        g = nc.scalar.activation(
                            y1[:, hf, :], pm[:, hf * P:(hf + 1) * P], AF.Gelu,
                            bias=bu1_t[:, hf:hf + 1])
                        if first_gelu[0] is None:
                            first_gelu[0] = g
                            if last_exp[0] is not None:
                                add_dep_helper(
                                    g.ins, last_exp[0].ins,
                                    reason="gelu after exp (ACT tables)")
                    po = ps_mm.tile([P, D], F32, tag="pmc")
                    for k in range(2):
                        nc.tensor.matmul(po[:], lhsT=y1[:, k, :],
                                         rhs=u2_t[:, k, :],
                                         start=(k == 0), stop=False)
                    nc.tensor.matmul(po[:], lhsT=ones_t[0:1, :],
                                     rhs=bu2_t[0:1, :], start=False, stop=True)
                    ot = op.tile([P, D], F32, tag="fin")
                    nc.scalar.activation(ot[:], po[:], AF.Gelu)
                    nc.sync.dma_start(out_dram[t * P:(t + 1) * P, :], ot[:])

            phase_c(NT_I, hT_i_in, msg_inv, out_inv)

            for t in range(NT_A):
                nu = hp.tile([P, D], F32, tag="nu_f")
                nc.sync.dma_start(nu[:], numer_sh[t * P:(t + 1) * P, :])
                se = sp.tile([P, H], F32, tag="se_f")
                nc.sync.dma_start(se[:], sumex_sh[t * P:(t + 1) * P, :])
                den = sp.tile([P, H], F32, tag="den")
                nc.vector.tensor_scalar(den[:], se[:], 1e-10, None, ALU.add)
                rec = sp.tile([P, H], F32, tag="rec")
                nc.vector.reciprocal(rec[:], den[:])
                msg = op.tile([P, D], BF16, tag="msg")
                nc.vector.tensor_tensor(
                    msg[:].rearrange("p (h k) -> p h k", h=H),
                    nu[:].rearrange("p (h k) -> p h k", h=H),
                    rec[:].unsqueeze(-1).to_broadcast((P, H, DK)), ALU.mult)
                nc.sync.dma_start(msg_ast[t * P:(t + 1) * P, :], msg[:])

            phase_c(NT_A, hT_a_in, msg_ast, out_ast)

    nc.compile()
    return nc


# ----------------------------------------------------------------------------
# Entry point
# ----------------------------------------------------------------------------

def kernel(inv_h, asset_h, inv_norm_w, asset_norm_w,
           m_w1, m_b1, m_w2, m_b2, Wq, Wk, Wv,
           u_w1, u_b1, u_w2, u_b2, edge_tgt, edge_src):
    global _LAST_EXEC_NS
    bf = ml_dtypes.bfloat16
    inv_h = np.asarray(inv_h, np.float32)
    asset_h = np.asarray(asset_h, np.float32)
    inv_norm_w = np.asarray(inv_norm_w, np.float32)
    asset_norm_w = np.asarray(asset_norm_w, np.float32)
    edge_tgt = np.asarray(edge_tgt).astype(np.int64)
    edge_src = np.asarray(edge_src).astype(np.int64)
    m_w1, m_b1 = np.asarray(m_w1, np.float32), np.asarray(m_b1, np.float32)
    m_w2, m_b2 = np.asarray(m_w2, np.float32), np.asarray(m_b2, np.float32)
    Wq, Wk, Wv = (np.asarray(x, np.float32) for x in (Wq, Wk, Wv))
    u_w1, u_b1 = np.asarray(u_w1, np.float32), np.asarray(u_b1, np.float32)
    u_w2, u_b2 = np.asarray(u_w2, np.float32), np.asarray(u_b2, np.float32)

    ast_row = (edge_src // ASH) * ASHP + (edge_src % ASH)
    inv_core = edge_tgt // ISH
    inv_loc = edge_tgt - inv_core * ISH

    meta1, C1, d1 = _bucket(
        core=inv_core, blk=inv_loc // P, n_blocks=NT_I,
        srcidx=ast_row, t128_all=inv_loc % P, nw=inv_norm_w)
    meta2, C2, d2 = _bucket(
        core=inv_core, blk=ast_row // P, n_blocks=NB2,
        srcidx=inv_loc, t128_all=ast_row % P, nw=asset_norm_w)

    nc = _build(meta1, C1, meta2, C2)

    w_qk_h = np.concatenate([Wq.reshape(2, P, D), Wk.reshape(2, P, D)], axis=2)
    common = {
        "w_m1": m_w1.reshape(2, P, D).astype(bf),
        "w_m2": m_w2.reshape(2, P, D).astype(bf),
        "w_qk": w_qk_h.astype(bf),
        "w_v": Wv.reshape(2, P, D).astype(bf),
        "w_u1": u_w1.reshape(4, P, D).astype(bf),
        "w_u2": u_w2.reshape(2, P, D).astype(bf),
        "b_m1": m_b1.reshape(2, P).T.copy(),
        "b_m2": m_b2.reshape(2, P).T.copy(),
        "b_u1": u_b1.reshape(2, P).T.copy(),
        "b_u2r": u_b2.reshape(1, D).astype(bf),
    }

    in_maps = []
    for c in range(NC):
        s1, t1, n1 = d1[c]
        s2, t2, n2 = d2[c]
        m = dict(common)
        m["hT_i_in"] = _hT(_pad_rows(inv_h[c * ISH:(c + 1) * ISH], ISHP), NT_I)
        m["hT_a_in"] = _hT(_pad_rows(asset_h[c * ASH:(c + 1) * ASH], ASHP), NT_A)
        m["d1_src"] = _wrap16(s1, C1)
        m["d1_nw"] = _colmajor(n1, C1, rep=H)
        m["d1_oh"] = _onehots(t1, C1)
        m["d2_src"] = _wrap16(s2, C2)
        m["d2_nw"] = _colmajor(n2, C2, rep=H)
        m["d2_oh"] = _onehots(t2, C2)
        in_maps.append(m)

    res = bass_utils.run_bass_kernel_spmd(
        nc, in_maps, core_ids=list(range(NC)), trace=True)
    _LAST_EXEC_NS = res.exec_time_ns

    inv_out = np.concatenate(
        [res.results[c]["out_inv"][:ISH] for c in range(NC)], axis=0)
    ast_out = np.concatenate(
        [res.results[c]["out_ast"][:ASH] for c in range(NC)], axis=0)
    return inv_out, ast_out


# revision 15
# speedup vs baseline: 1.6870x; 1.0784x over previous
"""Trainium2 Bass kernel for nn_MessagePassingLayer (bipartite GNN attention
message passing), distributed over 8 NeuronCores.

v5 design notes:
  - Node tables row-sharded 8 ways (inv 6250->6272 padded, asset 1250->1280).
  - dma_gather on TRN2 is descriptor-generation bound (~5.9ns/idx + 2.6us/call,
    independent of row bytes), so K|V are packed into one fp32 [N,512] row
    (one gather per edge total) and Q is never gathered: per target block the
    128 Q rows are densely loaded and Qe is formed on the PE with a
    host-precomputed transposed one-hot (exact in bf16).
  - Phase A: project local shard; FF runs feature-major off a host-supplied
    transposed bf16 copy of h, so there are no on-device transposes.  Small
    asset tables (kv_a fp32, q_a bf16) are AllGathered; the 51MB investor
    tables stay local (dir2 is shard-by-source + ReduceScatter of the segment
    stats instead).
  - Phase B per 128-edge chunk: gather KV[src]; Qe = oh_te @ Qblk (PE);
    qk = Qe*Ke (DVE, fp32); per-head reduce; ex = nw*exp(s/8) (max-term
    dropped -- exact to 1e-10 rel here: ex equals the reference's exactly
    when the segment max < 0, and the 1e-10 eps is negligible otherwise);
    exe=[ex*V | ex] in bf16; numer|sumex accumulated per target block by one
    one-hot matmul into fp32 PSUM; divide at block end.  Batched DVE ops
    amortize op overhead across 8-chunk gather batches.
  - dir2 partial numer/sumex (over all 10240 asset rows) are ReduceScattered.
  - Phase C: out = FF([h, msg]); msg transposed via hardware DMA-transpose
    (bf16); matmuls bf16, final gelu fp32 out.
  - gelu and exp live in different ACT table sets; an explicit dep keeps all
    phase-C gelus after the last phase-B exp to avoid table-reload thrash.
"""

import ml_dtypes
import numpy as np
from contextlib import ExitStack

import concourse.bass as bass
import concourse.tile as tile
from concourse.tile import add_dep_helper
from concourse import bacc, mybir
from concourse import bass_utils

F32 = mybir.dt.float32
BF16 = mybir.dt.bfloat16
I16 = mybir.dt.int16
AF = mybir.ActivationFunctionType
ALU = mybir.AluOpType

I_N, A_N, E_N = 50000, 10000, 200000
D, H, DK = 256, 4, 64
D2 = 2 * D
NC = 8
P = 128
ISH, ASH = I_N // NC, A_N // NC            # 6250, 1250
ISHP, ASHP = 6272, 1280
NT_I, NT_A = ISHP // P, ASHP // P          # 49, 10
NB2 = ASHP * NC // P                       # 80 global asset blocks
SUBB = 8                                   # chunks per gather call (1024 idx cap)

_LAST_EXEC_NS = None


# ----------------------------------------------------------------------------
# Host-side edge preparation
# ----------------------------------------------------------------------------

def _bucket(core, blk, n_blocks, srcidx, t128_all, nw):
    buckets = {}
    for c in range(NC):
        m_c = core == c
        for b in range(n_blocks):
            buckets[(c, b)] = np.nonzero(m_c & (blk == b))[0]
    meta = [max((len(buckets[(c, b)]) + P - 1) // P for c in range(NC))
            for b in range(n_blocks)]
    C = sum(meta)
    cores_out = []
    for c in range(NC):
        s16 = np.zeros(C * P, np.int64)
        t128 = np.full(C * P, -1, np.int64)
        nwv = np.zeros(C * P, np.float32)
        pos = 0
        for b in range(n_blocks):
            idx = buckets[(c, b)]
            n = len(idx)
            if meta[b] == 0:
                continue
            sl = slice(pos, pos + n)
            s16[sl] = srcidx[idx]
            t128[sl] = t128_all[idx]
            nwv[sl] = nw[idx]
            pos += meta[b] * P
        assert pos == C * P
        cores_out.append((s16, t128, nwv))
    return meta, C, cores_out


def _wrap16(flat_idx, C):
    assert flat_idx.max(initial=0) < 32768 and flat_idx.min(initial=0) >= 0
    w = flat_idx.astype(np.int16).reshape(C * 8, 16).T
    return np.tile(w, (8, 1)).copy()


def _colmajor(flat, C, rep=1):
    a = flat.reshape(C, P).T.copy()
    if rep == 1:
        return a
    return np.repeat(a[:, :, None], rep, axis=2).copy()


def _onehots(t128, C):
    """[C*128] targets (-1=pad) -> combined [C, 128, 256] = [oh_te | oh_et]."""
    bf = ml_dtypes.bfloat16
    t = t128.reshape(C, P, 1)
    oh_et = (t == np.arange(P).reshape(1, 1, P)).astype(bf)
    out = np.empty((C, P, 2 * P), bf)
    out[:, :, 0:P] = oh_et.transpose(0, 2, 1)
    out[:, :, P:2 * P] = oh_et
    return out


def _pad_rows(a, n):
    out = np.zeros((n, a.shape[1]), a.dtype)
    out[: a.shape[0]] = a
    return out


def _hT(h_pad, ntiles):
    """[N,256] f32 -> bf16 transposed-tile layout [N,256]:
    row (t*128+f), col (k*128+n) = h[t*128+n, k*128+f]."""
    bf = ml_dtypes.bfloat16
    x = h_pad.reshape(ntiles, P, 2, P)          # (t, n, k, f)
    x = x.transpose(0, 3, 2, 1)                 # (t, f, k, n)
    return np.ascontiguousarray(x.reshape(ntiles * P, D).astype(bf))


def _chunk_info(meta):
    info = []
    for b, k in enumerate(meta):
        for j in range(k):
            info.append((b, j == 0, j == k - 1))
    return info


# ----------------------------------------------------------------------------
# Device program
# ----------------------------------------------------------------------------

def _build(meta1, C1, meta2, C2):
    nc = bacc.Bacc("TRN2", target_bir_lowering=False, debug=False,
                   enable_asserts=True, num_devices=NC)

    hT_i_in = nc.dram_tensor("hT_i_in", [NT_I * P, D], BF16, kind="ExternalInput")
    hT_a_in = nc.dram_tensor("hT_a_in", [NT_A * P, D], BF16, kind="ExternalInput")
    w_m1 = nc.dram_tensor("w_m1", [2, P, D], BF16, kind="ExternalInput")
    w_m2 = nc.dram_tensor("w_m2", [2, P, D], BF16, kind="ExternalInput")
    w_qk = nc.dram_tensor("w_qk", [2, P, D2], BF16, kind="ExternalInput")
    w_v = nc.dram_tensor("w_v", [2, P, D], BF16, kind="ExternalInput")
    w_u1 = nc.dram_tensor("w_u1", [4, P, D], BF16, kind="ExternalInput")
    w_u2 = nc.dram_tensor("w_u2", [2, P, D], BF16, kind="ExternalInput")
    b_m1 = nc.dram_tensor("b_m1", [P, 2], F32, kind="ExternalInput")
    b_m2 = nc.dram_tensor("b_m2", [P, 2], F32, kind="ExternalInput")
    b_u1 = nc.dram_tensor("b_u1", [P, 2], F32, kind="ExternalInput")
    b_u2r = nc.dram_tensor("b_u2r", [1, D], BF16, kind="ExternalInput")

    d1_src = nc.dram_tensor("d1_src", [P, C1 * 8], I16, kind="ExternalInput")
    d1_nw = nc.dram_tensor("d1_nw", [P, C1, H], F32, kind="ExternalInput")
    d1_oh = nc.dram_tensor("d1_oh", [C1, P, 2 * P], BF16, kind="ExternalInput")
    d2_src = nc.dram_tensor("d2_src", [P, C2 * 8], I16, kind="ExternalInput")
    d2_nw = nc.dram_tensor("d2_nw", [P, C2, H], F32, kind="ExternalInput")
    d2_oh = nc.dram_tensor("d2_oh", [C2, P, 2 * P], BF16, kind="ExternalInput")

    out_inv = nc.dram_tensor("out_inv", [ISHP, D], F32, kind="ExternalOutput")
    out_ast = nc.dram_tensor("out_ast", [ASHP, D], F32, kind="ExternalOutput")

    info1 = _chunk_info(meta1)
    info2 = _chunk_info(meta2)

    with tile.TileContext(nc) as tc:
        with ExitStack() as ctx:
            wpool = ctx.enter_context(tc.tile_pool(name="w", bufs=1))
            hp = ctx.enter_context(tc.tile_pool(name="hp", bufs=3))
            tp = ctx.enter_context(tc.tile_pool(name="tp", bufs=3))
            op = ctx.enter_context(tc.tile_pool(name="op", bufs=3))
            gp = ctx.enter_context(tc.tile_pool(name="gp", bufs=2))
            sp = ctx.enter_context(tc.tile_pool(name="sp", bufs=2))
            ohp = ctx.enter_context(tc.tile_pool(name="ohp", bufs=4))
            qbp = ctx.enter_context(tc.tile_pool(name="qbp", bufs=2))
            ps_mm = ctx.enter_context(tc.tile_pool(name="ps_mm", bufs=1, space="PSUM"))
            ps_c = ctx.enter_context(tc.tile_pool(name="ps_c", bufs=3, space="PSUM"))
            ps_nu = ctx.enter_context(tc.tile_pool(name="ps_nu", bufs=2, space="PSUM"))
            ps_qe = ctx.enter_context(tc.tile_pool(name="ps_qe", bufs=2, space="PSUM"))
            dram = ctx.enter_context(tc.tile_pool(name="dram", bufs=1, space="DRAM"))

            ones_t = wpool.tile([1, P], BF16, tag="ones_t")
            nc.vector.memset(ones_t[:], 1.0)
            z256 = wpool.tile([P, D], F32, tag="z256")
            nc.vector.memset(z256[:], 0.0)
            z4 = wpool.tile([P, H], F32, tag="z4")
            nc.vector.memset(z4[:], 0.0)
            z256b = wpool.tile([P, D], BF16, tag="z256b")
            nc.vector.memset(z256b[:], 0.0)

            def load_w(dram_w, nk, nd, dt, tag):
                t = wpool.tile([P, nk, nd], dt, tag=tag)
                for k in range(nk):
                    nc.sync.dma_start(t[:, k, :], dram_w[k, :, :])
                return t

            m1_t = load_w(w_m1, 2, D, BF16, "wm1")
            m2_t = load_w(w_m2, 2, D, BF16, "wm2")
            qk_t = load_w(w_qk, 2, D2, BF16, "wqk")
            v_t = load_w(w_v, 2, D, BF16, "wv")
            u1_t = load_w(w_u1, 4, D, BF16, "wu1")
            u2_t = load_w(w_u2, 2, D, BF16, "wu2")

            def load_b(dram_b, tag):
                t = wpool.tile([P, 2], F32, tag=tag)
                nc.sync.dma_start(t[:], dram_b[:, :])
                return t

            bm1_t, bm2_t, bu1_t = load_b(b_m1, "bm1"), load_b(b_m2, "bm2"), load_b(b_u1, "bu1")
            bu2_t = wpool.tile([1, D], BF16, tag="bu2")
            nc.sync.dma_start(bu2_t[:], b_u2r[:, :])

            q_inv = dram.tile([ISHP, D], BF16, tag="q_inv")
            kv_inv = dram.tile([ISHP, D2], BF16, tag="kv_inv")
            q_a_sh = dram.tile([ASHP, D], BF16, tag="q_a_sh")
            kv_a_sh = dram.tile([ASHP, D2], BF16, tag="kv_a_sh")
            q_a_full = dram.tile([ASHP * NC, D], BF16, tag="q_a_full")
            kv_a_full = dram.tile([ASHP * NC, D2], BF16, tag="kv_a_full")
            numer_d = dram.tile([ASHP * NC, D], F32, tag="numer_d")
            sumex_d = dram.tile([ASHP * NC, H], F32, tag="sumex_d")
            numer_sh = dram.tile([ASHP, D], F32, tag="numer_sh")
            sumex_sh = dram.tile([ASHP, H], F32, tag="sumex_sh")
            msg_inv = dram.tile([ISHP, D], BF16, tag="msg_inv")
            msg_ast = dram.tile([ASHP, D], BF16, tag="msg_ast")

            # ================= Phase A =================
            def phase_a(hT_in, ntiles, q_dram, kv_dram):
                for t in range(ntiles):
                    hTb = tp.tile([P, 2, P], BF16, tag="hTb")
                    nc.sync.dma_start(hTb[:], hT_in[t * P:(t + 1) * P, :])

                    def ff_layer(inT, w_tile, b_tile, tag):
                        outT = tp.tile([P, 2, P], BF16, tag=tag)
                        for hf in range(2):
                            pm = ps_mm.tile([P, D2], F32, tag="pm")
                            for k in range(2):
                                nc.tensor.matmul(
                                    pm[:, 0:P],
                                    lhsT=w_tile[:, k, hf * P:(hf + 1) * P],
                                    rhs=inT[:, k, :],
                                    start=(k == 0), stop=(k == 1))
                            nc.scalar.activation(
                                outT[:, hf, :], pm[:, 0:P], AF.Gelu,
                                bias=b_tile[:, hf:hf + 1])
                        return outT

                    mT1 = ff_layer(hTb, m1_t, bm1_t, "mT1")
                    mT2 = ff_layer(mT1, m2_t, bm2_t, "mT2")

                    pqk = ps_mm.tile([P, D2], F32, tag="pm")
                    for k in range(2):
                        nc.tensor.matmul(pqk[:], lhsT=hTb[:, k, :],
                                         rhs=qk_t[:, k, :],
                                         start=(k == 0), stop=(k == 1))
                    oq = op.tile([P, D], BF16, tag="proj_q")
                    nc.vector.tensor_copy(oq[:], pqk[:, 0:D])
                    nc.sync.dma_start(q_dram[t * P:(t + 1) * P, :], oq[:])
                    ok_ = op.tile([P, D], BF16, tag="proj_k")
                    nc.vector.tensor_copy(ok_[:], pqk[:, D:D2])
                    nc.sync.dma_start(kv_dram[t * P:(t + 1) * P, 0:D], ok_[:])
                    pv = ps_mm.tile([P, D2], F32, tag="pm")
                    for k in range(2):
                        nc.tensor.matmul(pv[:, 0:D], lhsT=mT2[:, k, :],
                                         rhs=v_t[:, k, :],
                                         start=(k == 0), stop=(k == 1))
                    ov = op.tile([P, D], BF16, tag="proj_v")
                    nc.vector.tensor_copy(ov[:], pv[:, 0:D])
                    nc.sync.dma_start(kv_dram[t * P:(t + 1) * P, D:D2], ov[:])

            phase_a(hT_a_in, NT_A, q_a_sh, kv_a_sh)

            rg = [list(range(NC))]
            nc.gpsimd.collective_compute(
                "AllGather", ALU.bypass, replica_groups=rg,
                ins=[kv_a_sh.opt()], outs=[kv_a_full.opt()])
            nc.gpsimd.collective_compute(
                "AllGather", ALU.bypass, replica_groups=rg,
                ins=[q_a_sh.opt()], outs=[q_a_full.opt()])

            phase_a(hT_i_in, NT_I, q_inv, kv_inv)

            # ================= Phase B =================
            last_exp = [None]

            def phase_b(info, C, meta, src_sb, nw_sb, oh_dram,
                        q_tbl, kv_tbl, sink):
                numer = [None]
                qblk = None

                def flush(batch):
                    g0, n, ohg, exe = batch
                    for j in range(n):
                        c = g0 + j
                        blk, first, last = info[c]
                        if first:
                            numer[0] = ps_nu.tile([P, D + H], F32, tag="nu", name="nu")
                        nu_ps = numer[0]
                        nc.tensor.matmul(nu_ps[:], lhsT=ohg[:, j, P:2 * P],
                                         rhs=exe[:, j, :],
                                         start=first, stop=last)
                        if last:
                            rows = slice(blk * P, (blk + 1) * P)
                            if sink[0] == "msg":
                                den = sp.tile([P, H], F32, tag="den")
                                nc.vector.tensor_scalar(
                                    den[:], nu_ps[:, D:D + H], 1e-10, None,
                                    ALU.add)
                                rec = sp.tile([P, H], F32, tag="rec")
                                nc.vector.reciprocal(rec[:], den[:])
                                msg = op.tile([P, D], BF16, tag="msg")
                                nc.vector.tensor_tensor(
                                    msg[:].rearrange("p (h k) -> p h k", h=H),
                                    nu_ps[:, 0:D].rearrange("p (h k) -> p h k", h=H),
                                    rec[:].unsqueeze(-1).to_broadcast((P, H, DK)),
                                    ALU.mult)
                                nc.sync.dma_start(sink[1][rows, :], msg[:])
                            else:
                                nu_sb = op.tile([P, D], F32, tag="nu_sb")
                                nc.vector.tensor_copy(nu_sb[:], nu_ps[:, 0:D])
                                nc.sync.dma_start(sink[1][rows, :], nu_sb[:])
                                se_sb = sp.tile([P, H], F32, tag="se_sb")
                                nc.vector.tensor_copy(se_sb[:], nu_ps[:, D:D + H])
                                nc.sync.dma_start(sink[2][rows, :], se_sb[:])

                pend = None
                for g0 in range(0, C, SUBB):
                    n = min(SUBB, C - g0)
                    kvg = gp.tile([P, SUBB, D2], BF16, tag="kvg", bufs=4)
                    nc.gpsimd.dma_gather(
                        out_ap=kvg[:, 0:n, :], in_ap=kv_tbl,
                        idxs_ap=src_sb[:, g0 * 8:(g0 + n) * 8],
                        num_idxs=n * P, num_idxs_reg=n * P, elem_size=D2)
                    ohg = ohp.tile([P, SUBB, 2 * P], BF16, tag="ohg")
                    nc.sync.dma_start(
                        ohg[:, 0:n, :],
                        oh_dram[g0:g0 + n].transpose([1, 0, 2]))
                    qk = sp.tile([P, SUBB, D], F32, tag="qk", bufs=3)
                    for j in range(n):
                        c = g0 + j
                        blk, first, last = info[c]
                        if first:
                            qblk = qbp.tile([P, D], BF16, tag="qblk")
                            nc.sync.dma_start(
                                qblk[:], q_tbl[blk * P:(blk + 1) * P, :])
                        qe = ps_qe.tile([P, D], F32, tag="qe")
                        nc.tensor.matmul(qe[:], lhsT=ohg[:, j, 0:P], rhs=qblk[:],
                                         start=True, stop=True)
                        nc.vector.tensor_tensor(
                            qk[:, j, :], qe[:], kvg[:, j, 0:D], ALU.mult)
                    s4 = sp.tile([P, SUBB, H], F32, tag="s4")
                    nc.vector.tensor_reduce(
                        s4[:, 0:n, :],
                        qk[:, 0:n, :].rearrange("p c (h k) -> p c h k", h=H),
                        axis=mybir.AxisListType.X, op=ALU.add)
                    ex0 = sp.tile([P, SUBB, H], F32, tag="ex0")
                    last_exp[0] = nc.scalar.activation(
                        ex0[:, 0:n, :], s4[:, 0:n, :], AF.Exp, scale=0.125)
                    exb = sp.tile([P, SUBB, H], F32, tag="exb")
                    nc.vector.tensor_tensor(
                        exb[:, 0:n, :], ex0[:, 0:n, :], nw_sb[:, g0:g0 + n, :],
                        ALU.mult)
                    exe = sp.tile([P, SUBB, D + H], BF16, tag="exe", bufs=3)
                    nc.vector.tensor_tensor(
                        exe[:, 0:n, 0:D].rearrange("p c (h k) -> p c h k", h=H),
                        kvg[:, 0:n, D:D2].rearrange("p c (h k) -> p c h k", h=H),
                        exb[:, 0:n, :].unsqueeze(-1).to_broadcast((P, n, H, DK)),
                        ALU.mult)
                    nc.vector.tensor_copy(exe[:, 0:n, D:D + H], exb[:, 0:n, :])
                    if pend is not None:
                        flush(pend)
                    pend = (g0, n, ohg, exe)
                if pend is not None:
                    flush(pend)
                for b, k in enumerate(meta):
                    if k != 0:
                        continue
                    rows = slice(b * P, (b + 1) * P)
                    if sink[0] == "msg":
                        nc.sync.dma_start(sink[1][rows, :], z256b[:])
                    else:
                        nc.sync.dma_start(sink[1][rows, :], z256[:])
                        nc.sync.dma_start(sink[2][rows, :], z4[:])

            d1_src_sb = wpool.tile([P, C1 * 8], I16, tag="d1_src_sb")
            nc.sync.dma_start(d1_src_sb[:], d1_src[:, :])
            d1_nw_sb = wpool.tile([P, C1, H], F32, tag="d1_nw_sb")
            nc.sync.dma_start(d1_nw_sb[:], d1_nw[:, :, :])
            d2_src_sb = wpool.tile([P, C2 * 8], I16, tag="d2_src_sb")
            nc.sync.dma_start(d2_src_sb[:], d2_src[:, :])
            d2_nw_sb = wpool.tile([P, C2, H], F32, tag="d2_nw_sb")
            nc.sync.dma_start(d2_nw_sb[:], d2_nw[:, :, :])

            phase_b(info1, C1, meta1, d1_src_sb, d1_nw_sb, d1_oh,
                    q_inv[:, :], kv_a_full[:, :], ("msg", msg_inv))
            phase_b(info2, C2, meta2, d2_src_sb, d2_nw_sb, d2_oh,
                    q_a_full[:, :], kv_inv[:, :], ("acc", numer_d, sumex_d))

            nc.gpsimd.collective_compute(
                "ReduceScatter", ALU.add, replica_groups=rg,
                ins=[numer_d.opt()], outs=[numer_sh.opt()])
            nc.gpsimd.collective_compute(
                "ReduceScatter", ALU.add, replica_groups=rg,
                ins=[sumex_d.opt()], outs=[sumex_sh.opt()])

            # ================= Phase C =================
            first_gelu = [None]

            def phase_c(ntiles, hT_in, msg_dram, out_dram):
                for t in range(ntiles):
                    cat = tp.tile([P, 4, P], BF16, tag="cat")
                    nc.sync.dma_start(cat[:, 0:2, :], hT_in[t * P:(t + 1) * P, :])
                    nc.sync.dma_start_transpose(
                        cat[:, 2, :], msg_dram[t * P:(t + 1) * P, 0:P])
                    nc.scalar.dma_start_transpose(
                        cat[:, 3, :], msg_dram[t * P:(t + 1) * P, P:D])
                    y1 = tp.tile([P, 2, P], BF16, tag="y1")
                    pm = ps_c.tile([P, D], F32, tag="pmc")
                    for hf in range(2):
                        for k in range(4):
                            nc.tensor.matmul(
                                pm[:, hf * P:(hf + 1) * P],
                                lhsT=u1_t[:, k, hf * P:(hf + 1) * P],
                                rhs=cat[:, k, :],
                                start=(k == 0), stop=(k == 3))
                    for hf in range(2):
                        g = nc.scalar.activation(
                            y1[:, hf, :], pm[:, hf * P:(hf + 1) * P], AF.Gelu,
                            bias=bu1_t[:, hf:hf + 1])
                        if first_gelu[0] is None:
                            first_gelu[0] = g
                            if last_exp[0] is not None:
                                add_dep_helper(
                                    g.ins, last_exp[0].ins,
                                    reason="gelu after exp (ACT tables)")
                    po = ps_c.tile([P, D], F32, tag="pmc")
                    for k in range(2):
                        nc.tensor.matmul(po[:], lhsT=y1[:, k, :],
                                         rhs=u2_t[:, k, :],
                                         start=(k == 0), stop=False)
                    nc.tensor.matmul(po[:], lhsT=ones_t[0:1, :],
                                     rhs=bu2_t[0:1, :], start=False, stop=True)
                    ot = op.tile([P, D], F32, tag="fin")
                    nc.scalar.activation(ot[:], po[:], AF.Gelu)
                    nc.sync.dma_start(out_dram[t * P:(t + 1) * P, :], ot[:])

            phase_c(NT_I, hT_i_in, msg_inv, out_inv)

            for t in range(NT_A):
                nu = hp.tile([P, D], F32, tag="nu_f")
                nc.sync.dma_start(nu[:], numer_sh[t * P:(t + 1) * P, :])
                se = sp.tile([P, H], F32, tag="se_f")
                nc.sync.dma_start(se[:], sumex_sh[t * P:(t + 1) * P, :])
                den = sp.tile([P, H], F32, tag="den")
                nc.vector.tensor_scalar(den[:], se[:], 1e-10, None, ALU.add)
                rec = sp.tile([P, H], F32, tag="rec")
                nc.vector.reciprocal(rec[:], den[:])
                msg = op.tile([P, D], BF16, tag="msg")
                nc.vector.tensor_tensor(
                    msg[:].rearrange("p (h k) -> p h k", h=H),
                    nu[:].rearrange("p (h k) -> p h k", h=H),
                    rec[:].unsqueeze(-1).to_broadcast((P, H, DK)), ALU.mult)
                nc.sync.dma_start(msg_ast[t * P:(t + 1) * P, :], msg[:])

            phase_c(NT_A, hT_a_in, msg_ast, out_ast)

    nc.compile()
    return nc


# ----------------------------------------------------------------------------
# Entry point
# ----------------------------------------------------------------------------

def kernel(inv_h, asset_h, inv_norm_w, asset_norm_w,
           m_w1, m_b1, m_w2, m_b2, Wq, Wk, Wv,
           u_w1, u_b1, u_w2, u_b2, edge_tgt, edge_src):
    global _LAST_EXEC_NS
    bf = ml_dtypes.bfloat16
    inv_h = np.asarray(inv_h, np.float32)
    asset_h = np.asarray(asset_h, np.float32)
    inv_norm_w = np.asarray(inv_norm_w, np.float32)
    asset_norm_w = np.asarray(asset_norm_w, np.float32)
    edge_tgt = np.asarray(edge_tgt).astype(np.int64)
    edge_src = np.asarray(edge_src).astype(np.int64)
    m_w1, m_b1 = np.asarray(m_w1, np.float32), np.asarray(m_b1, np.float32)
    m_w2, m_b2 = np.asarray(m_w2, np.float32), np.asarray(m_b2, np.float32)
    Wq, Wk, Wv = (np.asarray(x, np.float32) for x in (Wq, Wk, Wv))
    u_w1, u_b1 = np.asarray(u_w1, np.float32), np.asarray(u_b1, np.float32)
    u_w2, u_b2 = np.asarray(u_w2, np.float32), np.asarray(u_b2, np.float32)

    ast_row = (edge_src // ASH) * ASHP + (edge_src % ASH)
    inv_core = edge_tgt // ISH
    inv_loc = edge_tgt - inv_core * ISH

    meta1, C1, d1 = _bucket(
        core=inv_core, blk=inv_loc // P, n_blocks=NT_I,
        srcidx=ast_row, t128_all=inv_loc % P, nw=inv_norm_w)
    meta2, C2, d2 = _bucket(
        core=inv_core, blk=ast_row // P, n_blocks=NB2,
        srcidx=inv_loc, t128_all=ast_row % P, nw=asset_norm_w)

    nc = _build(meta1, C1, meta2, C2)

    w_qk_h = np.concatenate([Wq.reshape(2, P, D), Wk.reshape(2, P, D)], axis=2)
    common = {
        "w_m1": m_w1.reshape(2, P, D).astype(bf),
        "w_m2": m_w2.reshape(2, P, D).astype(bf),
        "w_qk": w_qk_h.astype(bf),
        "w_v": Wv.reshape(2, P, D).astype(bf),
        "w_u1": u_w1.reshape(4, P, D).astype(bf),
        "w_u2": u_w2.reshape(2, P, D).astype(bf),
        "b_m1": m_b1.reshape(2, P).T.copy(),
        "b_m2": m_b2.reshape(2, P).T.copy(),
        "b_u1": u_b1.reshape(2, P).T.copy(),
        "b_u2r": u_b2.reshape(1, D).astype(bf),
    }

    in_maps = []
    for c in range(NC):
        s1, t1, n1 = d1[c]
        s2, t2, n2 = d2[c]
        m = dict(common)
        m["hT_i_in"] = _hT(_pad_rows(inv_h[c * ISH:(c + 1) * ISH], ISHP), NT_I)
        m["hT_a_in"] = _hT(_pad_rows(asset_h[c * ASH:(c + 1) * ASH], ASHP), NT_A)
        m["d1_src"] = _wrap16(s1, C1)
        m["d1_nw"] = _colmajor(n1, C1, rep=H)
        m["d1_oh"] = _onehots(t1, C1)
        m["d2_src"] = _wrap16(s2, C2)
        m["d2_nw"] = _colmajor(n2, C2, rep=H)
        m["d2_oh"] = _onehots(t2, C2)
        in_maps.append(m)

    res = bass_utils.run_bass_kernel_spmd(
        nc, in_maps, core_ids=list(range(NC)), trace=True)
    _LAST_EXEC_NS = res.exec_time_ns

    inv_out = np.concatenate(
        [res.results[c]["out_inv"][:ISH] for c in range(NC)], axis=0)
    ast_out = np.concatenate(
        [res.results[c]["out_ast"][:ASH] for c in range(NC)], axis=0)
    return inv_out, ast_out
